# revision 10
# baseline (speedup 1.0000x reference)
"""Trainium2 Bass kernel for nn_ConvBlock (SepGconv + LayerNorm + GELU MLP).

Computes, for full inputs:
    a   = einsum('bsc,brsd,dc->brc', x, kernel_basis, kernel_W) + conv_bias
    a   = LayerNorm(a) * ln_scale + ln_bias          (over channels, eps=1e-6)
    out = gelu_tanh(a @ W1 + b1) @ W2 + b2

Shapes: B=2, N=1024 (R=S=N), H=64, D=32, WF=4.

Sharding: the (B*R)=2048 output rows split into 8 contiguous shards of 256
rows, one per NeuronCore. Each core reads its kernel_basis shard once
(memory-bound), contracts over all S on-chip, and runs the LN/MLP tail
locally. x / weights are replicated.

Perf strategy (HBM-bound at ~360 GB/s per core):
- kernel_basis streams in PURE bf16 (2 B/elem, 16.8 MB/core, ~47 us of bus;
  the gate is rel_err < 2e-2 and bf16 lands ~2.5e-3). All 16 one-MB j-block
  tiles are prefetched into SBUF up front (SBUF is ~26 MB) so the queues
  never wait on buffer recycling.
- dma_start issue costs ~0.6 us of serial sequencer time each, so the kb
  stream is issued alone on the sync queue while x + all small constants
  ride in two packed-blob DMAs issued from the (otherwise idle) Scalar
  engine's queue.
- Each matmul is psum[c, (r,d)] += x[s,c]^T @ kb[s,(r,d)] with N=512
  (16 rows x 32 d), K=128 s-chunk; the d-reduction against kernel_W runs
  on DVE (multiply by W broadcast + free-axis reduce).
- LayerNorm stats are accumulated DURING the stream: per j-block DVE forms
  a+cb and its square, and a tiny ones-matmul drops their channel sums into
  a persistent PSUM tile. Row quarters then only need a short rsqrt/scale/
  MLP chain, staggered through the j-loop; ln_scale/ln_bias are folded into
  W1/b1 on the host so the tail chain is minimal. Only the last quarter's
  chain (~7 us) runs after the stream; the last two kb tiles are DMA'd
  per s-chunk so PE can start on partial tiles.
"""

import os

import numpy as np

import concourse.bass as bass
import concourse.tile as tile
from concourse import mybir
from concourse.bass_utils import run_bass_kernel_spmd


def _ensure_axon_hooks():
    """bass_utils imports antenv.axon_hooks when trace=True under axon; some
    images ship antenv without that module. Register a functional stand-in
    (driving NTFF capture via libaxon_pjrt.so) so tracing works, degrading
    to hook=None (no trace, run still works) if the .so is unavailable."""
    import sys
    import types

    try:
        import antenv.axon_hooks  # noqa: F401

        return
    except ImportError:
        pass
    try:
        import antenv
    except ImportError:
        antenv = types.ModuleType("antenv")
        sys.modules["antenv"] = antenv

    mod = types.ModuleType("antenv.axon_hooks")
    mod._hook = None

    def set_axon_ntff_profile_hook(h):
        mod._hook = h

    def get_axon_ntff_profile_hook():
        if mod._hook is None:
            try:
                from trn_agent_boot.trn_boot import _ntff_profile_via_ctypes

                so_path = "/opt/axon/libaxon_pjrt.so"
                if os.path.exists(so_path):
                    mod._hook = _ntff_profile_via_ctypes(so_path)
            except Exception:
                mod._hook = None
        return mod._hook

    mod.set_axon_ntff_profile_hook = set_axon_ntff_profile_hook
    mod.get_axon_ntff_profile_hook = get_axon_ntff_profile_hook
    sys.modules["antenv.axon_hooks"] = mod
    antenv.axon_hooks = mod


try:
    _ensure_axon_hooks()
except Exception:
    pass

F32 = mybir.dt.float32
BF16 = mybir.dt.bfloat16

B, N, H, D, WF = 2, 1024, 64, 32, 4
NCORES = 8
ROWS_PER_CORE = (B * N) // NCORES  # 256
RB = 16  # rows per j-block
N_JBLK = ROWS_PER_CORE // RB  # 16
N_KCHUNK = N // 128  # 8 s-chunks of 128
FH = WF * H  # 256
Q = ROWS_PER_CORE // 4  # 64 rows per tail quarter
LN_EPS = 1e-6
N_WARM = 7
NEWTON_ITERS = 2
KSPLIT_J = (14, 15)  # j-blocks whose kb DMA is issued per s-chunk

# consts blob column layout (f32 words)
BC_WB = 0  # [0:64, 0:512]   wb2: W[d,c] tiled per r
BC_W1 = 512  # [0:65, 512:768] [ln_scale*W1 ; ln_bias@W1+b1]
BC_W2 = 768  # [0:128, 768:896] W2 as [p, fh*64+h]
BC_B2 = 896  # [0:64, 896:960] b2 broadcast over rows
BC_CB = 960  # [0:64, 960:961] conv_bias
BC_W = 968

_NC_CACHE = None
LAST_EXEC_NS = None


def _build_nc(split_waits=True):
    nc = bass.Bass(target_bir_lowering=False)

    kbh = nc.dram_tensor("kbh", [N_JBLK, 128, N_KCHUNK, RB, D], BF16, kind="ExternalInput")
    xcp = nc.dram_tensor("xcp", [128, N_KCHUNK, H], BF16, kind="ExternalInput")
    blob = nc.dram_tensor("blob", [128, BC_W], F32, kind="ExternalInput")
    out = nc.dram_tensor("out", [ROWS_PER_CORE, H], F32, kind="ExternalOutput")

    with tile.TileContext(nc) as tc:
        with (
            tc.tile_pool(name="consts", bufs=1) as consts,
            tc.tile_pool(name="kbhp", bufs=N_JBLK) as kbh_pool,
            tc.tile_pool(name="mwp", bufs=2) as mw_pool,
            tc.tile_pool(name="tmpp", bufs=2) as tmp_pool,
            tc.tile_pool(name="work", bufs=2) as work,
            tc.tile_pool(name="pmain", bufs=3, space="PSUM") as pmain,
            tc.tile_pool(name="pstatp", bufs=1, space="PSUM") as pstatp,
            tc.tile_pool(name="ptail", bufs=3, space="PSUM") as ptail,
            tc.tile_pool(name="pwarm", bufs=1, space="PSUM") as pwarm,
        ):
            # ---- x + consts blob ride the Scalar engine's DMA queue so the
            # sync queue is kb-only and the stream starts immediately ----
            xc_sb = consts.tile([128, N_KCHUNK, H], BF16)
            nc.scalar.dma_start(out=xc_sb, in_=xcp[:, :, :])
            blob_sb = consts.tile([128, BC_W], F32)
            nc.scalar.dma_start(out=blob_sb, in_=blob[:, :])

            wb_sb = blob_sb[0:H, BC_WB : BC_WB + RB * D]
            w1_sb = blob_sb[0 : H + 1, BC_W1 : BC_W1 + FH]
            w2_sb = blob_sb[:, BC_W2 : BC_W2 + 2 * H]
            b2_sb = blob_sb[0:Q, BC_B2 : BC_B2 + H]
            cb_sb = blob_sb[0:H, BC_CB : BC_CB + 1]

            # ---- the kernel_basis stream: all 16 MB prefetched; the last
            # two j-blocks arrive per s-chunk so PE can chase the stream ----
            kb_tiles = {}
            for j0 in range(N_JBLK):
                t = kbh_pool.tile([128, N_KCHUNK, RB, D], BF16, name=f"kbh_t{j0}", tag="kbh_t")
                kb_tiles[j0] = t
                if j0 in KSPLIT_J:
                    for k in range(N_KCHUNK):
                        nc.sync.dma_start(out=t[:, k, :, :], in_=kbh[j0, :, k, :, :])
                else:
                    nc.sync.dma_start(out=t, in_=kbh[j0, :, :, :, :])

            # ---- small on-chip constants (GpSimd, idle otherwise) ----
            ones64 = consts.tile([H, 1], F32)
            nc.gpsimd.memset(ones64, 1.0)
            ones1 = consts.tile([1, H], F32)
            nc.gpsimd.memset(ones1, 1.0)
            z_sb = consts.tile([H + 1, Q], F32)
            nc.gpsimd.memset(z_sb[H : H + 1, :], 1.0)
            rp = consts.tile([1, 2 * Q], F32)
            stv = consts.tile([H, 2, ROWS_PER_CORE], F32)  # [a+cb ; (a+cb)^2]

            # ---- PE warm-up on the x tile while kb j0 is in flight (HAM
            # needs ~3us of continuous activity to reach 2.4 GHz) ----
            ps_warm = pwarm.tile([128, RB * D], F32)
            for w in range(N_WARM):
                nc.tensor.matmul(
                    ps_warm[0:H, :],
                    lhsT=xc_sb[:, 0, :],
                    rhs=xc_sb.rearrange("p a b -> p (a b)")[:, 0 : RB * D],
                    start=True,
                    stop=True,
                )

            # persistent LN-stats accumulator: [1, j, (sum(a+cb), sum((a+cb)^2)), r]
            pstat = pstatp.tile([1, N_JBLK, 2, RB], F32)

            # ---- tail pieces per row-quarter, staggered through the j-loop
            # so every engine-queue entry's inputs are long-ready ----
            state = {}

            def t_qt(q):
                sl4 = slice(4 * q, 4 * (q + 1))
                qt = work.tile([1, Q], F32, name=f"qt{q}", tag="qt")
                nc.vector.tensor_scalar(
                    out=qt, in0=pstat[:, sl4, 1, :], scalar1=1.0 / H, scalar2=LN_EPS,
                    op0=mybir.AluOpType.mult, op1=mybir.AluOpType.add,
                )
                mu = work.tile([1, Q], F32, name=f"mu{q}", tag="mu")
                nc.vector.tensor_scalar(
                    out=mu, in0=pstat[:, sl4, 0, :], scalar1=-1.0 / H, scalar2=None,
                    op0=mybir.AluOpType.mult,
                )
                t3 = work.tile([1, Q], F32, name=f"t3_{q}", tag="t3")
                nc.vector.tensor_mul(t3, mu, mu)
                nc.vector.tensor_sub(qt, qt, t3)
                state[("qt", q)] = qt
                state[("mu", q)] = mu

            def t_newton(q):
                qt = state[("qt", q)]
                mu = state[("mu", q)]
                # rsqrt on DVE only (ScalarE's LUT stays pinned on gelu):
                # quake seed via int<->float value casts + Newton steps.
                uf = work.tile([1, Q], F32, name=f"uf{q}", tag="uf")
                nc.vector.tensor_copy(out=uf, in_=qt.bitcast(mybir.dt.int32))
                nc.vector.tensor_scalar(
                    out=uf, in0=uf, scalar1=-0.5, scalar2=float(0x5F3759DF),
                    op0=mybir.AluOpType.mult, op1=mybir.AluOpType.add,
                )
                yi = work.tile([1, Q], mybir.dt.int32, name=f"yi{q}", tag="yi")
                nc.vector.tensor_copy(out=yi, in_=uf)
                y = yi.bitcast(F32)
                t1 = work.tile([1, Q], F32, name=f"t1_{q}", tag="t1")
                for it in range(NEWTON_ITERS):
                    nc.vector.tensor_mul(t1, y, y)
                    nc.vector.tensor_mul(t1, t1, qt)
                    nc.vector.tensor_scalar(
                        out=t1, in0=t1, scalar1=-0.5, scalar2=1.5,
                        op0=mybir.AluOpType.mult, op1=mybir.AluOpType.add,
                    )
                    if it == NEWTON_ITERS - 1:
                        nc.vector.tensor_mul(rp[:, 0:Q], y, t1)
                    else:
                        nc.vector.tensor_mul(y, y, t1)
                nc.vector.tensor_mul(rp[:, Q : 2 * Q], rp[:, 0:Q], mu)

            def t_bc(q):
                ps_bc = ptail.tile([H, 2 * Q], F32, name=f"ps_bc{q}", tag="ps_bc", bufs=1)
                nc.tensor.matmul(ps_bc, lhsT=ones1, rhs=rp, start=True, stop=True)
                nc.vector.tensor_mul(
                    z_sb[0:H, :], stv[:, 0, Q * q : Q * (q + 1)], ps_bc[:, 0:Q]
                )
                nc.vector.tensor_add(z_sb[0:H, :], z_sb[0:H, :], ps_bc[:, Q : 2 * Q])

            def t_mlp(q):
                ph = ptail.tile([128, 2, Q], F32, name=f"ph{q}", tag="ph", bufs=1)
                for fh in range(2):
                    nc.tensor.matmul(
                        ph[:, fh, :],
                        lhsT=w1_sb[:, 128 * fh : 128 * (fh + 1)],
                        rhs=z_sb,
                        start=True,
                        stop=True,
                    )
                hT = work.tile([128, 2, Q], F32, name=f"hT{q}", tag="hT")
                nc.scalar.activation(
                    out=hT.rearrange("p a b -> p (a b)"),
                    in_=ph.rearrange("p a b -> p (a b)"),
                    func=mybir.ActivationFunctionType.Gelu_apprx_tanh,
                    bias=0.0,
                    scale=1.0,
                )
                po = ptail.tile([Q, H], F32, name=f"po{q}", tag="po", bufs=1)
                for fh in range(2):
                    nc.tensor.matmul(
                        po,
                        lhsT=hT[:, fh, :],
                        rhs=w2_sb[:, H * fh : H * (fh + 1)],
                        start=(fh == 0),
                        stop=(fh == 1),
                    )
                o_sb = work.tile([Q, H], F32, name=f"o_sb{q}", tag="o_sb")
                nc.vector.tensor_add(o_sb, po, b2_sb)
                nc.sync.dma_start(out=out[Q * q : Q * (q + 1), :], in_=o_sb)

            sched = {}
            for q in range(3):
                sched[4 * q + 4] = (lambda q=q: t_qt(q),)
                sched[4 * q + 5] = (lambda q=q: t_newton(q),)
                sched[4 * q + 6] = (lambda q=q: t_bc(q),)
                sched[4 * q + 7] = (lambda q=q: t_mlp(q),)

            # ---- main contraction ----
            for j in range(N_JBLK):
                kbh_t = kb_tiles.pop(j)
                ps = pmain.tile([H, RB * D], F32)
                for k in range(N_KCHUNK):
                    nc.tensor.matmul(
                        ps, lhsT=xc_sb[:, k, :], rhs=kbh_t[:, k, :, :],
                        start=(k == 0), stop=(k == N_KCHUNK - 1),
                    )
                mw = mw_pool.tile([H, RB, D], F32)
                nc.vector.tensor_mul(
                    mw.rearrange("p a b -> p (a b)"), ps, wb_sb
                )
                tmp = tmp_pool.tile([H, RB], F32)
                nc.vector.tensor_reduce(
                    out=tmp,
                    in_=mw,
                    axis=mybir.AxisListType.X,
                    op=mybir.AluOpType.add,
                )
                st0 = stv[:, 0, RB * j : RB * (j + 1)]
                nc.vector.tensor_scalar(
                    out=st0, in0=tmp, scalar1=cb_sb, scalar2=None,
                    op0=mybir.AluOpType.add,
                )
                nc.vector.tensor_mul(stv[:, 1, RB * j : RB * (j + 1)], st0, st0)
                nc.tensor.matmul(
                    pstat[:, j, :, :],
                    lhsT=ones64,
                    rhs=stv[:, :, RB * j : RB * (j + 1)],
                    start=True,
                    stop=True,
                )
                for fn in sched.get(j, ()):
                    fn()

            # last quarter after the stream
            t_qt(3)
            t_newton(3)
            t_bc(3)
            t_mlp(3)

    if split_waits:
        _split_matmul_waits(nc)
    return nc


def _split_matmul_waits(nc):
    """This walrus build rejects engine instructions carrying more than one
    semaphore wait ("Too many sync wait commands"). Peel all but the last
    wait off onto same-engine NoOps inserted immediately before the
    instruction — NoOps execute in queue order on the same sequencer, so the
    wait semantics are unchanged."""
    f = nc.m.functions[0]
    nop_id = 0
    for blk in f.blocks:
        insts = list(blk.instructions)
        out = []
        changed = False
        for inst in insts:
            si = inst.sync_info
            if (
                si is not None
                and si.on_wait is not None
                and len(si.on_wait) > 1
                and getattr(inst, "engine", None) is not None
            ):
                waits = list(si.on_wait)
                for w in waits[:-1]:
                    nop = mybir.InstNoOp(
                        name=f"I-mmwait-{nop_id}",
                        engine=inst.engine,
                        ins=[],
                        outs=[],
                        sync_info=mybir.SyncInfo(on_wait=[w], on_update=[]),
                    )
                    nop_id += 1
                    out.append(nop)
                inst.sync_info = mybir.SyncInfo(
                    on_wait=[waits[-1]], on_update=list(si.on_update or [])
                )
                changed = True
            out.append(inst)
        if changed:
            blk.instructions = out
    return nc


def _get_nc():
    global _NC_CACHE
    if _NC_CACHE is None:
        _NC_CACHE = _build_nc()
    return _NC_CACHE


def _prep_blob(kernel_W, conv_bias, ln_scale, ln_bias, W1, b1, W2, b2):
    blob = np.zeros((128, BC_W), np.float32)
    # wb2[c, r*D + d] = W[d, c]
    blob[0:H, BC_WB : BC_WB + RB * D] = np.tile(kernel_W.T, (1, RB))
    # ln_scale folded into W1 rows; ln_bias@W1 + b1 as the K=65 bias row
    blob[0:H, BC_W1 : BC_W1 + FH] = W1 * ln_scale[:, None]
    blob[H, BC_W1 : BC_W1 + FH] = ln_bias @ W1 + b1
    blob[:, BC_W2 : BC_W2 + 2 * H] = W2.reshape(2, 128, H).transpose(1, 0, 2).reshape(128, 2 * H)
    blob[0:Q, BC_B2 : BC_B2 + H] = np.broadcast_to(b2, (Q, H))
    blob[0:H, BC_CB] = conv_bias
    return blob


def _prep_x(xb):
    # (N, H) -> (128, k, H) bf16, with s = 128*k + p
    import ml_dtypes

    xh = xb.astype(ml_dtypes.bfloat16)
    return np.ascontiguousarray(xh.reshape(N_KCHUNK, 128, H).transpose(1, 0, 2))


def _prep_kb_shard(shard):
    # shard (256, 1024, 32) -> (j, p, k, r^, d) bf16
    import ml_dtypes

    hi = shard.astype(ml_dtypes.bfloat16)
    return np.ascontiguousarray(
        hi.reshape(N_JBLK, RB, N_KCHUNK, 128, D).transpose(0, 3, 2, 1, 4)
    )


def kernel(
    x,
    kernel_basis,
    kernel_W,
    conv_bias,
    ln_scale,
    ln_bias,
    W1,
    b1,
    W2,
    b2,
):
    global LAST_EXEC_NS
    x = np.ascontiguousarray(np.asarray(x, np.float32))
    kb = np.ascontiguousarray(np.asarray(kernel_basis, np.float32))
    blob = _prep_blob(
        np.asarray(kernel_W, np.float32),
        np.asarray(conv_bias, np.float32),
        np.asarray(ln_scale, np.float32),
        np.asarray(ln_bias, np.float32),
        np.asarray(W1, np.float32),
        np.asarray(b1, np.float32),
        np.asarray(W2, np.float32),
        np.asarray(b2, np.float32),
    )
    xps = [_prep_x(x[b]) for b in range(B)]

    kbf = kb.reshape(B * N, N, D)
    in_maps = []
    for c in range(NCORES):
        hi = _prep_kb_shard(kbf[c * ROWS_PER_CORE : (c + 1) * ROWS_PER_CORE])
        in_maps.append(dict(kbh=hi, xcp=xps[c // (NCORES // B)], blob=blob))

    nc = _get_nc()
    trace = bool(os.environ.get("KERNEL_BASS_TRACE"))
    res = run_bass_kernel_spmd(nc, in_maps, core_ids=list(range(NCORES)), trace=trace)
    LAST_EXEC_NS = res.exec_time_ns

    outs = np.concatenate([res.results[c]["out"] for c in range(NCORES)], axis=0)
    return outs.reshape(B, N, H)


# revision 11
# speedup vs baseline: 1.1599x; 1.1599x over previous
"""Trainium2 Bass kernel for nn_ConvBlock (SepGconv + LayerNorm + GELU MLP).

Computes, for full inputs:
    a   = einsum('bsc,brsd,dc->brc', x, kernel_basis, kernel_W) + conv_bias
    a   = LayerNorm(a) * ln_scale + ln_bias          (over channels, eps=1e-6)
    out = gelu_tanh(a @ W1 + b1) @ W2 + b2

Shapes: B=2, N=1024 (R=S=N), H=64, D=32, WF=4.

Sharding: the (B*R)=2048 output rows split into 8 contiguous shards of 256
rows, one per NeuronCore. Each core reads its kernel_basis shard once
(memory-bound), contracts over all S on-chip, and runs the LN/MLP tail
locally. x / weights are replicated.

Perf strategy (HBM-bound at ~360 GB/s per core):
- kernel_basis streams in PURE bf16 (2 B/elem, 16.8 MB/core, ~47 us of bus;
  the gate is rel_err < 2e-2 and bf16 lands ~2.5e-3). All 16 one-MB j-block
  tiles are prefetched into SBUF up front (SBUF is ~26 MB) so the queues
  never wait on buffer recycling.
- dma_start issue costs ~0.6 us of serial sequencer time each, so the kb
  stream is issued alone on the sync queue while x + all small constants
  ride in two packed-blob DMAs issued from the (otherwise idle) Scalar
  engine's queue.
- Each matmul is psum[c, (r,d)] += x[s,c]^T @ kb[s,(r,d)] with N=512
  (16 rows x 32 d), K=128 s-chunk; the d-reduction against kernel_W runs
  on DVE (multiply by W broadcast + free-axis reduce).
- LayerNorm stats are accumulated DURING the stream: per j-block DVE forms
  a+cb and its square, and a tiny ones-matmul drops their channel sums into
  a persistent PSUM tile. Row quarters then only need a short rsqrt/scale/
  MLP chain, staggered through the j-loop; ln_scale/ln_bias are folded into
  W1/b1 on the host so the tail chain is minimal. Only the last quarter's
  chain (~7 us) runs after the stream; the last two kb tiles are DMA'd
  per s-chunk so PE can start on partial tiles.
"""

import os

import numpy as np

import concourse.bass as bass
import concourse.tile as tile
from concourse import mybir
from concourse.bass_utils import run_bass_kernel_spmd


def _ensure_axon_hooks():
    """bass_utils imports antenv.axon_hooks when trace=True under axon; some
    images ship antenv without that module. Register a functional stand-in
    (driving NTFF capture via libaxon_pjrt.so) so tracing works, degrading
    to hook=None (no trace, run still works) if the .so is unavailable."""
    import sys
    import types

    try:
        import antenv.axon_hooks  # noqa: F401

        return
    except ImportError:
        pass
    try:
        import antenv
    except ImportError:
        antenv = types.ModuleType("antenv")
        sys.modules["antenv"] = antenv

    mod = types.ModuleType("antenv.axon_hooks")
    mod._hook = None

    def set_axon_ntff_profile_hook(h):
        mod._hook = h

    def get_axon_ntff_profile_hook():
        if mod._hook is None:
            try:
                from trn_agent_boot.trn_boot import _ntff_profile_via_ctypes

                so_path = "/opt/axon/libaxon_pjrt.so"
                if os.path.exists(so_path):
                    mod._hook = _ntff_profile_via_ctypes(so_path)
            except Exception:
                mod._hook = None
        return mod._hook

    mod.set_axon_ntff_profile_hook = set_axon_ntff_profile_hook
    mod.get_axon_ntff_profile_hook = get_axon_ntff_profile_hook
    sys.modules["antenv.axon_hooks"] = mod
    antenv.axon_hooks = mod


try:
    _ensure_axon_hooks()
except Exception:
    pass

F32 = mybir.dt.float32
BF16 = mybir.dt.bfloat16

B, N, H, D, WF = 2, 1024, 64, 32, 4
NCORES = 8
ROWS_PER_CORE = (B * N) // NCORES  # 256
RB = 16  # rows per j-block
N_JBLK = ROWS_PER_CORE // RB  # 16
N_KCHUNK = N // 128  # 8 s-chunks of 128
FH = WF * H  # 256
Q = ROWS_PER_CORE // 4  # 64 rows per tail quarter
LN_EPS = 1e-6
N_WARM = 7
NEWTON_ITERS = 2
KSPLIT_J = (14, 15)  # j-blocks whose kb DMA is issued per s-chunk

# consts blob column layout (f32 words)
BC_WB = 0  # [0:64, 0:512]   wb2: W[d,c] tiled per r
BC_W1 = 512  # [0:65, 512:768] [ln_scale*W1 ; ln_bias@W1+b1]
BC_W2 = 768  # [0:128, 768:896] W2 as [p, fh*64+h]
BC_B2 = 896  # [0:64, 896:960] b2 broadcast over rows
BC_CB = 960  # [0:64, 960:961] conv_bias
BC_W = 968

_NC_CACHE = None
LAST_EXEC_NS = None


def _build_nc(split_waits=True):
    nc = bass.Bass(target_bir_lowering=False)

    kbh = nc.dram_tensor("kbh", [N_JBLK, 128, N_KCHUNK, RB, D], BF16, kind="ExternalInput")
    xcp = nc.dram_tensor("xcp", [128, N_KCHUNK, H], BF16, kind="ExternalInput")
    blob = nc.dram_tensor("blob", [128, BC_W], F32, kind="ExternalInput")
    out = nc.dram_tensor("out", [ROWS_PER_CORE, H], F32, kind="ExternalOutput")

    with tile.TileContext(nc) as tc:
        with (
            tc.tile_pool(name="consts", bufs=1) as consts,
            tc.tile_pool(name="kbhp", bufs=N_JBLK) as kbh_pool,
            tc.tile_pool(name="mwp", bufs=2) as mw_pool,
            tc.tile_pool(name="tmpp", bufs=2) as tmp_pool,
            tc.tile_pool(name="work", bufs=2) as work,
            tc.tile_pool(name="pmain", bufs=3, space="PSUM") as pmain,
            tc.tile_pool(name="pstatp", bufs=1, space="PSUM") as pstatp,
            tc.tile_pool(name="ptail", bufs=3, space="PSUM") as ptail,
            tc.tile_pool(name="pwarm", bufs=1, space="PSUM") as pwarm,
        ):
            # ---- x + consts blob ride the Scalar engine's DMA queue so the
            # sync queue is kb-only and the stream starts immediately ----
            xc_sb = consts.tile([128, N_KCHUNK, H], BF16)
            nc.scalar.dma_start(out=xc_sb, in_=xcp[:, :, :])
            blob_sb = consts.tile([128, BC_W], F32)
            nc.scalar.dma_start(out=blob_sb, in_=blob[:, :])

            wb_sb = blob_sb[0:H, BC_WB : BC_WB + RB * D]
            w1_sb = blob_sb[0 : H + 1, BC_W1 : BC_W1 + FH]
            w2_sb = blob_sb[:, BC_W2 : BC_W2 + 2 * H]
            b2_sb = blob_sb[0:Q, BC_B2 : BC_B2 + H]
            cb_sb = blob_sb[0:H, BC_CB : BC_CB + 1]

            # ---- the kernel_basis stream: all 16 MB prefetched; the last
            # two j-blocks arrive per s-chunk so PE can chase the stream ----
            kb_tiles = {}
            for j0 in range(N_JBLK):
                t = kbh_pool.tile([128, N_KCHUNK, RB, D], BF16, name=f"kbh_t{j0}", tag="kbh_t")
                kb_tiles[j0] = t
                if j0 in KSPLIT_J:
                    for k in range(N_KCHUNK):
                        nc.sync.dma_start(out=t[:, k, :, :], in_=kbh[j0, :, k, :, :])
                else:
                    nc.sync.dma_start(out=t, in_=kbh[j0, :, :, :, :])

            # ---- small on-chip constants (GpSimd, idle otherwise) ----
            ones64 = consts.tile([H, 1], F32)
            nc.gpsimd.memset(ones64, 1.0)
            ones1 = consts.tile([1, H], F32)
            nc.gpsimd.memset(ones1, 1.0)
            z_sb = consts.tile([H + 1, Q], F32)
            nc.gpsimd.memset(z_sb[H : H + 1, :], 1.0)
            rp = consts.tile([1, 2 * Q], F32)
            stv = consts.tile([H, 2, ROWS_PER_CORE], F32)  # [a+cb ; (a+cb)^2]

            # ---- PE warm-up on the x tile while kb j0 is in flight (HAM
            # needs ~3us of continuous activity to reach 2.4 GHz) ----
            ps_warm = pwarm.tile([128, RB * D], F32)
            for w in range(N_WARM):
                nc.tensor.matmul(
                    ps_warm[0:H, :],
                    lhsT=xc_sb[:, 0, :],
                    rhs=xc_sb.rearrange("p a b -> p (a b)")[:, 0 : RB * D],
                    start=True,
                    stop=True,
                )

            # persistent LN-stats accumulator: [1, j, (sum(a+cb), sum((a+cb)^2)), r]
            pstat = pstatp.tile([1, N_JBLK, 2, RB], F32)

            # ---- tail pieces per row-quarter, staggered through the j-loop
            # so every engine-queue entry's inputs are long-ready ----
            state = {}

            def t_qt(q):
                sl4 = slice(4 * q, 4 * (q + 1))
                qt = work.tile([1, Q], F32, name=f"qt{q}", tag="qt")
                nc.vector.tensor_scalar(
                    out=qt, in0=pstat[:, sl4, 1, :], scalar1=1.0 / H, scalar2=LN_EPS,
                    op0=mybir.AluOpType.mult, op1=mybir.AluOpType.add,
                )
                mu = work.tile([1, Q], F32, name=f"mu{q}", tag="mu")
                nc.vector.tensor_scalar(
                    out=mu, in0=pstat[:, sl4, 0, :], scalar1=-1.0 / H, scalar2=None,
                    op0=mybir.AluOpType.mult,
                )
                t3 = work.tile([1, Q], F32, name=f"t3_{q}", tag="t3")
                nc.vector.tensor_mul(t3, mu, mu)
                nc.vector.tensor_sub(qt, qt, t3)
                state[("qt", q)] = qt
                state[("mu", q)] = mu

            def t_newton(q):
                qt = state[("qt", q)]
                mu = state[("mu", q)]
                # rsqrt on DVE only (ScalarE's LUT stays pinned on gelu):
                # quake seed via int<->float value casts + Newton steps.
                uf = work.tile([1, Q], F32, name=f"uf{q}", tag="uf")
                nc.vector.tensor_copy(out=uf, in_=qt.bitcast(mybir.dt.int32))
                nc.vector.tensor_scalar(
                    out=uf, in0=uf, scalar1=-0.5, scalar2=float(0x5F3759DF),
                    op0=mybir.AluOpType.mult, op1=mybir.AluOpType.add,
                )
                yi = work.tile([1, Q], mybir.dt.int32, name=f"yi{q}", tag="yi")
                nc.vector.tensor_copy(out=yi, in_=uf)
                y = yi.bitcast(F32)
                t1 = work.tile([1, Q], F32, name=f"t1_{q}", tag="t1")
                for it in range(NEWTON_ITERS):
                    nc.vector.tensor_mul(t1, y, y)
                    nc.vector.tensor_mul(t1, t1, qt)
                    nc.vector.tensor_scalar(
                        out=t1, in0=t1, scalar1=-0.5, scalar2=1.5,
                        op0=mybir.AluOpType.mult, op1=mybir.AluOpType.add,
                    )
                    if it == NEWTON_ITERS - 1:
                        nc.vector.tensor_mul(rp[:, 0:Q], y, t1)
                    else:
                        nc.vector.tensor_mul(y, y, t1)
                nc.vector.tensor_mul(rp[:, Q : 2 * Q], rp[:, 0:Q], mu)

            def t_bc(q):
                ps_bc = ptail.tile([H, 2 * Q], F32, name=f"ps_bc{q}", tag="ps_bc", bufs=1)
                nc.tensor.matmul(ps_bc, lhsT=ones1, rhs=rp, start=True, stop=True)
                nc.vector.tensor_mul(
                    z_sb[0:H, :], stv[:, 0, Q * q : Q * (q + 1)], ps_bc[:, 0:Q]
                )
                nc.vector.tensor_add(z_sb[0:H, :], z_sb[0:H, :], ps_bc[:, Q : 2 * Q])

            def t_mlp(q):
                ph = ptail.tile([128, 2, Q], F32, name=f"ph{q}", tag="ph", bufs=1)
                for fh in range(2):
                    nc.tensor.matmul(
                        ph[:, fh, :],
                        lhsT=w1_sb[:, 128 * fh : 128 * (fh + 1)],
                        rhs=z_sb,
                        start=True,
                        stop=True,
                    )
                hT = work.tile([128, 2, Q], F32, name=f"hT{q}", tag="hT")
                nc.scalar.activation(
                    out=hT.rearrange("p a b -> p (a b)"),
                    in_=ph.rearrange("p a b -> p (a b)"),
                    func=mybir.ActivationFunctionType.Gelu_apprx_tanh,
                    bias=0.0,
                    scale=1.0,
                )
                po = ptail.tile([Q, H], F32, name=f"po{q}", tag="po", bufs=1)
                for fh in range(2):
                    nc.tensor.matmul(
                        po,
                        lhsT=hT[:, fh, :],
                        rhs=w2_sb[:, H * fh : H * (fh + 1)],
                        start=(fh == 0),
                        stop=(fh == 1),
                    )
                o_sb = work.tile([Q, H], F32, name=f"o_sb{q}", tag="o_sb")
                nc.vector.tensor_add(o_sb, po, b2_sb)
                nc.sync.dma_start(out=out[Q * q : Q * (q + 1), :], in_=o_sb)

            sched = {}
            for q in range(3):
                sched[4 * q + 4] = (lambda q=q: t_qt(q),)
                sched[4 * q + 5] = (lambda q=q: t_newton(q),)
                sched[4 * q + 6] = (lambda q=q: t_bc(q),)
                sched[4 * q + 7] = (lambda q=q: t_mlp(q),)

            def stats_mm(j):
                # emitted one j-block late so its DVE-produced inputs are
                # long-ready when the in-order PE queue reaches it
                nc.tensor.matmul(
                    pstat[:, j, :, :],
                    lhsT=ones64,
                    rhs=stv[:, :, RB * j : RB * (j + 1)],
                    start=True,
                    stop=True,
                )

            # ---- main contraction ----
            for j in range(N_JBLK):
                kbh_t = kb_tiles.pop(j)
                ps = pmain.tile([H, RB * D], F32)
                for k in range(N_KCHUNK):
                    nc.tensor.matmul(
                        ps, lhsT=xc_sb[:, k, :], rhs=kbh_t[:, k, :, :],
                        start=(k == 0), stop=(k == N_KCHUNK - 1),
                    )
                if j > 0:
                    stats_mm(j - 1)
                mw = mw_pool.tile([H, RB, D], F32)
                nc.vector.tensor_mul(
                    mw.rearrange("p a b -> p (a b)"), ps, wb_sb
                )
                tmp = tmp_pool.tile([H, RB], F32)
                nc.vector.tensor_reduce(
                    out=tmp,
                    in_=mw,
                    axis=mybir.AxisListType.X,
                    op=mybir.AluOpType.add,
                )
                st0 = stv[:, 0, RB * j : RB * (j + 1)]
                nc.vector.tensor_scalar(
                    out=st0, in0=tmp, scalar1=cb_sb, scalar2=None,
                    op0=mybir.AluOpType.add,
                )
                nc.vector.tensor_mul(stv[:, 1, RB * j : RB * (j + 1)], st0, st0)
                for fn in sched.get(j, ()):
                    fn()

            # last quarter after the stream
            stats_mm(N_JBLK - 1)
            t_qt(3)
            t_newton(3)
            t_bc(3)
            t_mlp(3)

    if split_waits:
        _split_matmul_waits(nc)
    return nc


def _split_matmul_waits(nc):
    """This walrus build rejects engine instructions carrying more than one
    semaphore wait ("Too many sync wait commands"). Peel all but the last
    wait off onto same-engine NoOps inserted immediately before the
    instruction — NoOps execute in queue order on the same sequencer, so the
    wait semantics are unchanged."""
    f = nc.m.functions[0]
    nop_id = 0
    for blk in f.blocks:
        insts = list(blk.instructions)
        out = []
        changed = False
        for inst in insts:
            si = inst.sync_info
            if (
                si is not None
                and si.on_wait is not None
                and len(si.on_wait) > 1
                and getattr(inst, "engine", None) is not None
            ):
                waits = list(si.on_wait)
                for w in waits[:-1]:
                    nop = mybir.InstNoOp(
                        name=f"I-mmwait-{nop_id}",
                        engine=inst.engine,
                        ins=[],
                        outs=[],
                        sync_info=mybir.SyncInfo(on_wait=[w], on_update=[]),
                    )
                    nop_id += 1
                    out.append(nop)
                inst.sync_info = mybir.SyncInfo(
                    on_wait=[waits[-1]], on_update=list(si.on_update or [])
                )
                changed = True
            out.append(inst)
        if changed:
            blk.instructions = out
    return nc


def _get_nc():
    global _NC_CACHE
    if _NC_CACHE is None:
        _NC_CACHE = _build_nc()
    return _NC_CACHE


def _prep_blob(kernel_W, conv_bias, ln_scale, ln_bias, W1, b1, W2, b2):
    blob = np.zeros((128, BC_W), np.float32)
    # wb2[c, r*D + d] = W[d, c]
    blob[0:H, BC_WB : BC_WB + RB * D] = np.tile(kernel_W.T, (1, RB))
    # ln_scale folded into W1 rows; ln_bias@W1 + b1 as the K=65 bias row
    blob[0:H, BC_W1 : BC_W1 + FH] = W1 * ln_scale[:, None]
    blob[H, BC_W1 : BC_W1 + FH] = ln_bias @ W1 + b1
    blob[:, BC_W2 : BC_W2 + 2 * H] = W2.reshape(2, 128, H).transpose(1, 0, 2).reshape(128, 2 * H)
    blob[0:Q, BC_B2 : BC_B2 + H] = np.broadcast_to(b2, (Q, H))
    blob[0:H, BC_CB] = conv_bias
    return blob


def _prep_x(xb):
    # (N, H) -> (128, k, H) bf16, with s = 128*k + p
    import ml_dtypes

    xh = xb.astype(ml_dtypes.bfloat16)
    return np.ascontiguousarray(xh.reshape(N_KCHUNK, 128, H).transpose(1, 0, 2))


def _prep_kb_shard(shard):
    # shard (256, 1024, 32) -> (j, p, k, r^, d) bf16
    import ml_dtypes

    hi = shard.astype(ml_dtypes.bfloat16)
    return np.ascontiguousarray(
        hi.reshape(N_JBLK, RB, N_KCHUNK, 128, D).transpose(0, 3, 2, 1, 4)
    )


def kernel(
    x,
    kernel_basis,
    kernel_W,
    conv_bias,
    ln_scale,
    ln_bias,
    W1,
    b1,
    W2,
    b2,
):
    global LAST_EXEC_NS
    x = np.ascontiguousarray(np.asarray(x, np.float32))
    kb = np.ascontiguousarray(np.asarray(kernel_basis, np.float32))
    blob = _prep_blob(
        np.asarray(kernel_W, np.float32),
        np.asarray(conv_bias, np.float32),
        np.asarray(ln_scale, np.float32),
        np.asarray(ln_bias, np.float32),
        np.asarray(W1, np.float32),
        np.asarray(b1, np.float32),
        np.asarray(W2, np.float32),
        np.asarray(b2, np.float32),
    )
    xps = [_prep_x(x[b]) for b in range(B)]

    kbf = kb.reshape(B * N, N, D)
    in_maps = []
    for c in range(NCORES):
        hi = _prep_kb_shard(kbf[c * ROWS_PER_CORE : (c + 1) * ROWS_PER_CORE])
        in_maps.append(dict(kbh=hi, xcp=xps[c // (NCORES // B)], blob=blob))

    nc = _get_nc()
    trace = bool(os.environ.get("KERNEL_BASS_TRACE"))
    res = run_bass_kernel_spmd(nc, in_maps, core_ids=list(range(NCORES)), trace=trace)
    LAST_EXEC_NS = res.exec_time_ns

    outs = np.concatenate([res.results[c]["out"] for c in range(NCORES)], axis=0)
    return outs.reshape(B, N, H)


# revision 19
# speedup vs baseline: 1.1792x; 1.0166x over previous
"""Trainium2 Bass kernel for nn_ConvBlock (SepGconv + LayerNorm + GELU MLP).

Computes, for full inputs:
    a   = einsum('bsc,brsd,dc->brc', x, kernel_basis, kernel_W) + conv_bias
    a   = LayerNorm(a) * ln_scale + ln_bias          (over channels, eps=1e-6)
    out = gelu_tanh(a @ W1 + b1) @ W2 + b2

Shapes: B=2, N=1024 (R=S=N), H=64, D=32, WF=4.

Sharding: the (B*R)=2048 output rows split into 8 contiguous shards of 256
rows, one per NeuronCore. Each core reads its kernel_basis shard once
(memory-bound), contracts over all S on-chip, and runs the LN/MLP tail
locally. x / weights are replicated.

Perf strategy (HBM-bound at ~360 GB/s per core):
- kernel_basis streams in PURE bf16 (2 B/elem, 16.8 MB/core, ~47 us of bus;
  the gate is rel_err < 2e-2 and bf16 lands ~2.5e-3). All 16 one-MB j-block
  tiles are prefetched into SBUF up front (SBUF is ~26 MB) so the queues
  never wait on buffer recycling.
- dma_start issue costs ~0.6 us of serial sequencer time each, so the kb
  stream is issued alone on the sync queue while x + all small constants
  ride in two packed-blob DMAs issued from the (otherwise idle) Scalar
  engine's queue.
- Each matmul is psum[c, (r,d)] += x[s,c]^T @ kb[s,(r,d)] with N=512
  (16 rows x 32 d), K=128 s-chunk; the d-reduction against kernel_W runs
  on DVE (multiply by W broadcast + free-axis reduce).
- LayerNorm stats are accumulated DURING the stream: per j-block DVE forms
  a+cb and its square, and a tiny ones-matmul drops their channel sums into
  a persistent PSUM tile. Row quarters then only need a short rsqrt/scale/
  MLP chain, staggered through the j-loop; ln_scale/ln_bias are folded into
  W1/b1 on the host so the tail chain is minimal. Only the last quarter's
  chain (~7 us) runs after the stream; the last two kb tiles are DMA'd
  per s-chunk so PE can start on partial tiles.
"""

import os

import numpy as np

import concourse.bass as bass
import concourse.tile as tile
from concourse import mybir
from concourse.bass_utils import run_bass_kernel_spmd


def _ensure_axon_hooks():
    """bass_utils imports antenv.axon_hooks when trace=True under axon; some
    images ship antenv without that module. Register a functional stand-in
    (driving NTFF capture via libaxon_pjrt.so) so tracing works, degrading
    to hook=None (no trace, run still works) if the .so is unavailable."""
    import sys
    import types

    try:
        import antenv.axon_hooks  # noqa: F401

        return
    except ImportError:
        pass
    try:
        import antenv
    except ImportError:
        antenv = types.ModuleType("antenv")
        sys.modules["antenv"] = antenv

    mod = types.ModuleType("antenv.axon_hooks")
    mod._hook = None

    def set_axon_ntff_profile_hook(h):
        mod._hook = h

    def get_axon_ntff_profile_hook():
        if mod._hook is None:
            try:
                from trn_agent_boot.trn_boot import _ntff_profile_via_ctypes

                so_path = "/opt/axon/libaxon_pjrt.so"
                if os.path.exists(so_path):
                    mod._hook = _ntff_profile_via_ctypes(so_path)
            except Exception:
                mod._hook = None
        return mod._hook

    mod.set_axon_ntff_profile_hook = set_axon_ntff_profile_hook
    mod.get_axon_ntff_profile_hook = get_axon_ntff_profile_hook
    sys.modules["antenv.axon_hooks"] = mod
    antenv.axon_hooks = mod


try:
    _ensure_axon_hooks()
except Exception:
    pass

F32 = mybir.dt.float32
BF16 = mybir.dt.bfloat16

B, N, H, D, WF = 2, 1024, 64, 32, 4
NCORES = 8
ROWS_PER_CORE = (B * N) // NCORES  # 256
RB = 16  # rows per j-block
N_JBLK = ROWS_PER_CORE // RB  # 16
N_KCHUNK = N // 128  # 8 s-chunks of 128
FH = WF * H  # 256
Q = ROWS_PER_CORE // 4  # 64 rows per tail quarter
LN_EPS = 1e-6
N_WARM = 7
NEWTON_ITERS = 1
KSPLIT_J = (14, 15)  # j-blocks whose kb DMA is issued per s-chunk

# consts blob column layout (f32 words)
BC_WB = 0  # [0:64, 0:512]   wb2: W[d,c] tiled per r
BC_W1 = 512  # [0:65, 512:768] [ln_scale*W1 ; ln_bias@W1+b1]
BC_W2 = 768  # [0:128, 768:896] W2 as [p, fh*64+h]
BC_B2 = 896  # [0:64, 896:960] b2 broadcast over rows
BC_CB = 960  # [0:64, 960:961] conv_bias
BC_W = 968

_NC_CACHE = None
LAST_EXEC_NS = None


def _build_nc(split_waits=True):
    nc = bass.Bass(target_bir_lowering=False)

    kbh = nc.dram_tensor("kbh", [N_JBLK, 128, N_KCHUNK, RB, D], BF16, kind="ExternalInput")
    xcp = nc.dram_tensor("xcp", [128, N_KCHUNK, H], BF16, kind="ExternalInput")
    blob = nc.dram_tensor("blob", [128, BC_W], F32, kind="ExternalInput")
    out = nc.dram_tensor("out", [ROWS_PER_CORE, H], F32, kind="ExternalOutput")

    with tile.TileContext(nc) as tc:
        with (
            tc.tile_pool(name="consts", bufs=1) as consts,
            tc.tile_pool(name="kbhp", bufs=N_JBLK) as kbh_pool,
            tc.tile_pool(name="mwp", bufs=2) as mw_pool,
            tc.tile_pool(name="tmpp", bufs=2) as tmp_pool,
            tc.tile_pool(name="work", bufs=2) as work,
            tc.tile_pool(name="pmain", bufs=3, space="PSUM") as pmain,
            tc.tile_pool(name="pstatp", bufs=1, space="PSUM") as pstatp,
            tc.tile_pool(name="ptail", bufs=3, space="PSUM") as ptail,
            tc.tile_pool(name="pwarm", bufs=1, space="PSUM") as pwarm,
        ):
            # ---- x + consts blob ride the Scalar engine's DMA queue so the
            # sync queue is kb-only and the stream starts immediately ----
            xc_sb = consts.tile([128, N_KCHUNK, H], BF16)
            nc.scalar.dma_start(out=xc_sb, in_=xcp[:, :, :])
            blob_sb = consts.tile([128, BC_W], F32)
            nc.scalar.dma_start(out=blob_sb, in_=blob[:, :])

            wb_sb = blob_sb[0:H, BC_WB : BC_WB + RB * D]
            w1_sb = blob_sb[0 : H + 1, BC_W1 : BC_W1 + FH]
            w2_sb = blob_sb[:, BC_W2 : BC_W2 + 2 * H]
            b2_sb = blob_sb[0:Q, BC_B2 : BC_B2 + H]
            cb_sb = blob_sb[0:H, BC_CB : BC_CB + 1]

            # ---- the kernel_basis stream: all 16 MB prefetched; the last
            # two j-blocks arrive per s-chunk so PE can chase the stream ----
            kb_tiles = {}
            for j0 in range(N_JBLK):
                t = kbh_pool.tile([128, N_KCHUNK, RB, D], BF16, name=f"kbh_t{j0}", tag="kbh_t")
                kb_tiles[j0] = t
                if j0 in KSPLIT_J:
                    for k in range(N_KCHUNK):
                        nc.sync.dma_start(out=t[:, k, :, :], in_=kbh[j0, :, k, :, :])
                else:
                    nc.sync.dma_start(out=t, in_=kbh[j0, :, :, :, :])

            # ---- small on-chip constants (GpSimd, idle otherwise) ----
            ones64 = consts.tile([H, 1], F32)
            nc.gpsimd.memset(ones64, 1.0)
            ones1 = consts.tile([1, H], F32)
            nc.gpsimd.memset(ones1, 1.0)
            z_sb = consts.tile([H + 1, Q], F32)
            nc.gpsimd.memset(z_sb[H : H + 1, :], 1.0)
            rp = consts.tile([1, 2 * Q], F32)
            stv = consts.tile([H, 2, ROWS_PER_CORE], F32)  # [a+cb ; (a+cb)^2]

            # ---- PE warm-up on the x tile while kb j0 is in flight (HAM
            # needs ~3us of continuous activity to reach 2.4 GHz) ----
            ps_warm = pwarm.tile([128, RB * D], F32)
            for w in range(N_WARM):
                nc.tensor.matmul(
                    ps_warm[0:H, :],
                    lhsT=xc_sb[:, 0, :],
                    rhs=xc_sb.rearrange("p a b -> p (a b)")[:, 0 : RB * D],
                    start=True,
                    stop=True,
                )

            # persistent LN-stats accumulator: [1, j, (sum(a+cb), sum((a+cb)^2)), r]
            pstat = pstatp.tile([1, N_JBLK, 2, RB], F32)

            # ---- tail pieces per row-quarter, staggered through the j-loop
            # so every engine-queue entry's inputs are long-ready ----
            state = {}

            def t_qt(q):
                sl4 = slice(4 * q, 4 * (q + 1))
                qt = work.tile([1, Q], F32, name=f"qt{q}", tag="qt")
                nc.vector.tensor_scalar(
                    out=qt, in0=pstat[:, sl4, 1, :], scalar1=1.0 / H, scalar2=LN_EPS,
                    op0=mybir.AluOpType.mult, op1=mybir.AluOpType.add,
                )
                mu = work.tile([1, Q], F32, name=f"mu{q}", tag="mu")
                nc.vector.tensor_scalar(
                    out=mu, in0=pstat[:, sl4, 0, :], scalar1=-1.0 / H, scalar2=None,
                    op0=mybir.AluOpType.mult,
                )
                t3 = work.tile([1, Q], F32, name=f"t3_{q}", tag="t3")
                nc.vector.tensor_mul(t3, mu, mu)
                nc.vector.tensor_sub(qt, qt, t3)
                state[("qt", q)] = qt
                state[("mu", q)] = mu

            def t_newton(q):
                qt = state[("qt", q)]
                mu = state[("mu", q)]
                # rsqrt on DVE only (ScalarE's LUT stays pinned on gelu):
                # quake seed via int<->float value casts + Newton steps.
                uf = work.tile([1, Q], F32, name=f"uf{q}", tag="uf")
                nc.vector.tensor_copy(out=uf, in_=qt.bitcast(mybir.dt.int32))
                nc.vector.tensor_scalar(
                    out=uf, in0=uf, scalar1=-0.5, scalar2=float(0x5F3759DF),
                    op0=mybir.AluOpType.mult, op1=mybir.AluOpType.add,
                )
                yi = work.tile([1, Q], mybir.dt.int32, name=f"yi{q}", tag="yi")
                nc.vector.tensor_copy(out=yi, in_=uf)
                y = yi.bitcast(F32)
                t1 = work.tile([1, Q], F32, name=f"t1_{q}", tag="t1")
                for it in range(NEWTON_ITERS):
                    nc.vector.tensor_mul(t1, y, y)
                    nc.vector.tensor_mul(t1, t1, qt)
                    nc.vector.tensor_scalar(
                        out=t1, in0=t1, scalar1=-0.5, scalar2=1.5,
                        op0=mybir.AluOpType.mult, op1=mybir.AluOpType.add,
                    )
                    if it == NEWTON_ITERS - 1:
                        nc.vector.tensor_mul(rp[:, 0:Q], y, t1)
                    else:
                        nc.vector.tensor_mul(y, y, t1)
                nc.vector.tensor_mul(rp[:, Q : 2 * Q], rp[:, 0:Q], mu)

            def t_bc(q):
                ps_bc = ptail.tile([H, 2 * Q], F32, name=f"ps_bc{q}", tag="ps_bc", bufs=1)
                nc.tensor.matmul(ps_bc, lhsT=ones1, rhs=rp, start=True, stop=True)
                nc.vector.tensor_mul(
                    z_sb[0:H, :], stv[:, 0, Q * q : Q * (q + 1)], ps_bc[:, 0:Q]
                )
                nc.vector.tensor_add(z_sb[0:H, :], z_sb[0:H, :], ps_bc[:, Q : 2 * Q])

            def t_mlp_a(q):
                ph = ptail.tile([128, 2, Q], F32, name=f"ph{q}", tag="ph", bufs=1)
                for fh in range(2):
                    nc.tensor.matmul(
                        ph[:, fh, :],
                        lhsT=w1_sb[:, 128 * fh : 128 * (fh + 1)],
                        rhs=z_sb,
                        start=True,
                        stop=True,
                    )
                hT = work.tile([128, 2, Q], F32, name=f"hT{q}", tag="hT")
                nc.scalar.activation(
                    out=hT.rearrange("p a b -> p (a b)"),
                    in_=ph.rearrange("p a b -> p (a b)"),
                    func=mybir.ActivationFunctionType.Gelu_apprx_tanh,
                    bias=0.0,
                    scale=1.0,
                )
                state[("hT", q)] = hT

            def t_mlp_b(q):
                hT = state[("hT", q)]
                po = ptail.tile([Q, H], F32, name=f"po{q}", tag="po", bufs=1)
                for fh in range(2):
                    nc.tensor.matmul(
                        po,
                        lhsT=hT[:, fh, :],
                        rhs=w2_sb[:, H * fh : H * (fh + 1)],
                        start=(fh == 0),
                        stop=(fh == 1),
                    )
                o_sb = work.tile([Q, H], F32, name=f"o_sb{q}", tag="o_sb")
                nc.vector.tensor_add(o_sb, po, b2_sb)
                nc.sync.dma_start(out=out[Q * q : Q * (q + 1), :], in_=o_sb)

            sched = {}
            for q in range(3):
                sched.setdefault(4 * q + 4, []).append(lambda q=q: t_qt(q))
                sched.setdefault(4 * q + 5, []).append(lambda q=q: t_newton(q))
                sched.setdefault(4 * q + 6, []).append(lambda q=q: t_bc(q))
                if 4 * q + 7 < N_JBLK - 1:
                    sched.setdefault(4 * q + 7, []).append(lambda q=q: t_mlp_a(q))
                if 4 * q + 8 < N_JBLK:
                    # runs ahead of the same slot's qt so the out DMA fires asap
                    sched.setdefault(4 * q + 8, []).insert(0, lambda q=q: t_mlp_b(q))

            def stats_mm(j):
                # emitted one j-block late so its DVE-produced inputs are
                # long-ready when the in-order PE queue reaches it
                nc.tensor.matmul(
                    pstat[:, j, :, :],
                    lhsT=ones64,
                    rhs=stv[:, :, RB * j : RB * (j + 1)],
                    start=True,
                    stop=True,
                )

            # ---- main contraction ----
            for j in range(N_JBLK):
                kbh_t = kb_tiles.pop(j)
                if j == N_JBLK - 1:
                    # PE has slack here (stream-gated); pulling this forward
                    # keeps the post-stream PE path minimal
                    stats_mm(j - 1)
                ps = pmain.tile([H, RB * D], F32)
                for k in range(N_KCHUNK):
                    nc.tensor.matmul(
                        ps, lhsT=xc_sb[:, k, :], rhs=kbh_t[:, k, :, :],
                        start=(k == 0), stop=(k == N_KCHUNK - 1),
                    )
                if 0 < j < N_JBLK - 1:
                    stats_mm(j - 1)
                mw = mw_pool.tile([H, RB, D], F32)
                nc.vector.tensor_mul(
                    mw.rearrange("p a b -> p (a b)"), ps, wb_sb
                )
                tmp = tmp_pool.tile([H, RB], F32)
                nc.vector.tensor_reduce(
                    out=tmp,
                    in_=mw,
                    axis=mybir.AxisListType.X,
                    op=mybir.AluOpType.add,
                )
                st0 = stv[:, 0, RB * j : RB * (j + 1)]
                nc.vector.tensor_scalar(
                    out=st0, in0=tmp, scalar1=cb_sb, scalar2=None,
                    op0=mybir.AluOpType.add,
                )
                nc.vector.tensor_mul(stv[:, 1, RB * j : RB * (j + 1)], st0, st0)
                for fn in sched.get(j, ()):
                    fn()

            # last quarter after the stream; q2's MLP rides under q3's
            # DVE chain
            stats_mm(N_JBLK - 1)
            t_qt(3)
            t_mlp_a(2)
            t_newton(3)
            t_mlp_b(2)
            t_bc(3)
            t_mlp_a(3)
            t_mlp_b(3)

    if split_waits:
        _split_matmul_waits(nc)
    return nc


def _split_matmul_waits(nc):
    """This walrus build rejects engine instructions carrying more than one
    semaphore wait ("Too many sync wait commands"). Peel all but the last
    wait off onto same-engine NoOps inserted immediately before the
    instruction — NoOps execute in queue order on the same sequencer, so the
    wait semantics are unchanged."""
    f = nc.m.functions[0]
    nop_id = 0
    for blk in f.blocks:
        insts = list(blk.instructions)
        out = []
        changed = False
        for inst in insts:
            si = inst.sync_info
            if (
                si is not None
                and si.on_wait is not None
                and len(si.on_wait) > 1
                and getattr(inst, "engine", None) is not None
            ):
                waits = list(si.on_wait)
                for w in waits[:-1]:
                    nop = mybir.InstNoOp(
                        name=f"I-mmwait-{nop_id}",
                        engine=inst.engine,
                        ins=[],
                        outs=[],
                        sync_info=mybir.SyncInfo(on_wait=[w], on_update=[]),
                    )
                    nop_id += 1
                    out.append(nop)
                inst.sync_info = mybir.SyncInfo(
                    on_wait=[waits[-1]], on_update=list(si.on_update or [])
                )
                changed = True
            out.append(inst)
        if changed:
            blk.instructions = out
    return nc


def _get_nc():
    global _NC_CACHE
    if _NC_CACHE is None:
        _NC_CACHE = _build_nc()
    return _NC_CACHE


def _prep_blob(kernel_W, conv_bias, ln_scale, ln_bias, W1, b1, W2, b2):
    blob = np.zeros((128, BC_W), np.float32)
    # wb2[c, r*D + d] = W[d, c]
    blob[0:H, BC_WB : BC_WB + RB * D] = np.tile(kernel_W.T, (1, RB))
    # ln_scale folded into W1 rows; ln_bias@W1 + b1 as the K=65 bias row
    blob[0:H, BC_W1 : BC_W1 + FH] = W1 * ln_scale[:, None]
    blob[H, BC_W1 : BC_W1 + FH] = ln_bias @ W1 + b1
    blob[:, BC_W2 : BC_W2 + 2 * H] = W2.reshape(2, 128, H).transpose(1, 0, 2).reshape(128, 2 * H)
    blob[0:Q, BC_B2 : BC_B2 + H] = np.broadcast_to(b2, (Q, H))
    blob[0:H, BC_CB] = conv_bias
    return blob


def _prep_x(xb):
    # (N, H) -> (128, k, H) bf16, with s = 128*k + p
    import ml_dtypes

    xh = xb.astype(ml_dtypes.bfloat16)
    return np.ascontiguousarray(xh.reshape(N_KCHUNK, 128, H).transpose(1, 0, 2))


def _prep_kb_shard(shard):
    # shard (256, 1024, 32) -> (j, p, k, r^, d) bf16
    import ml_dtypes

    hi = shard.astype(ml_dtypes.bfloat16)
    return np.ascontiguousarray(
        hi.reshape(N_JBLK, RB, N_KCHUNK, 128, D).transpose(0, 3, 2, 1, 4)
    )


def kernel(
    x,
    kernel_basis,
    kernel_W,
    conv_bias,
    ln_scale,
    ln_bias,
    W1,
    b1,
    W2,
    b2,
):
    global LAST_EXEC_NS
    x = np.ascontiguousarray(np.asarray(x, np.float32))
    kb = np.ascontiguousarray(np.asarray(kernel_basis, np.float32))
    blob = _prep_blob(
        np.asarray(kernel_W, np.float32),
        np.asarray(conv_bias, np.float32),
        np.asarray(ln_scale, np.float32),
        np.asarray(ln_bias, np.float32),
        np.asarray(W1, np.float32),
        np.asarray(b1, np.float32),
        np.asarray(W2, np.float32),
        np.asarray(b2, np.float32),
    )
    xps = [_prep_x(x[b]) for b in range(B)]

    kbf = kb.reshape(B * N, N, D)
    in_maps = []
    for c in range(NCORES):
        hi = _prep_kb_shard(kbf[c * ROWS_PER_CORE : (c + 1) * ROWS_PER_CORE])
        in_maps.append(dict(kbh=hi, xcp=xps[c // (NCORES // B)], blob=blob))

    nc = _get_nc()
    trace = bool(os.environ.get("KERNEL_BASS_TRACE"))
    res = run_bass_kernel_spmd(nc, in_maps, core_ids=list(range(NCORES)), trace=trace)
    LAST_EXEC_NS = res.exec_time_ns

    outs = np.concatenate([res.results[c]["out"] for c in range(NCORES)], axis=0)
    return outs.reshape(B, N, H)


# revision 27
# speedup vs baseline: 1.2826x; 1.0877x over previous
"""Trainium2 Bass kernel for nn_ConvBlock (SepGconv + LayerNorm + GELU MLP).

Computes, for full inputs:
    a   = einsum('bsc,brsd,dc->brc', x, kernel_basis, kernel_W) + conv_bias
    a   = LayerNorm(a) * ln_scale + ln_bias          (over channels, eps=1e-6)
    out = gelu_tanh(a @ W1 + b1) @ W2 + b2

Shapes: B=2, N=1024 (R=S=N), H=64, D=32, WF=4.

Sharding: the (B*R)=2048 output rows split into 8 contiguous shards of 256
rows, one per NeuronCore. Each core reads its kernel_basis shard once
(memory-bound), contracts over all S on-chip, and runs the LN/MLP tail
locally. x / weights are replicated.

Perf strategy (HBM-bound at ~360 GB/s per core):
- kernel_basis streams in PURE bf16 (2 B/elem, 16.8 MB/core, ~47 us of bus;
  the gate is rel_err < 2e-2 and bf16 lands ~2.5e-3). All 16 one-MB j-block
  tiles are prefetched into SBUF up front (SBUF is ~26 MB) so the queues
  never wait on buffer recycling.
- dma_start issue costs ~0.6 us of serial sequencer time each, so the kb
  stream is issued alone on the sync queue while x + all small constants
  ride in two packed-blob DMAs issued from the (otherwise idle) Scalar
  engine's queue.
- Each matmul is psum[c, (r,d)] += x[s,c]^T @ kb[s,(r,d)] with N=512
  (16 rows x 32 d), K=128 s-chunk; the d-reduction against kernel_W runs
  on DVE (multiply by W broadcast + free-axis reduce).
- LayerNorm stats are accumulated DURING the stream: per j-block DVE forms
  a+cb and its square, and a tiny ones-matmul drops their channel sums into
  a persistent PSUM tile. Row quarters then only need a short rsqrt/scale/
  MLP chain, staggered through the j-loop; ln_scale/ln_bias are folded into
  W1/b1 on the host so the tail chain is minimal. Only the last quarter's
  chain (~7 us) runs after the stream; the last two kb tiles are DMA'd
  per s-chunk so PE can start on partial tiles.
"""

import os

import numpy as np

import concourse.bass as bass
import concourse.tile as tile
from concourse import mybir
from concourse.bass_utils import run_bass_kernel_spmd


def _ensure_axon_hooks():
    """bass_utils imports antenv.axon_hooks when trace=True under axon; some
    images ship antenv without that module. Register a functional stand-in
    (driving NTFF capture via libaxon_pjrt.so) so tracing works, degrading
    to hook=None (no trace, run still works) if the .so is unavailable."""
    import sys
    import types

    try:
        import antenv.axon_hooks  # noqa: F401

        return
    except ImportError:
        pass
    try:
        import antenv
    except ImportError:
        antenv = types.ModuleType("antenv")
        sys.modules["antenv"] = antenv

    mod = types.ModuleType("antenv.axon_hooks")
    mod._hook = None

    def set_axon_ntff_profile_hook(h):
        mod._hook = h

    def get_axon_ntff_profile_hook():
        if mod._hook is None:
            try:
                from trn_agent_boot.trn_boot import _ntff_profile_via_ctypes

                so_path = "/opt/axon/libaxon_pjrt.so"
                if os.path.exists(so_path):
                    mod._hook = _ntff_profile_via_ctypes(so_path)
            except Exception:
                mod._hook = None
        return mod._hook

    mod.set_axon_ntff_profile_hook = set_axon_ntff_profile_hook
    mod.get_axon_ntff_profile_hook = get_axon_ntff_profile_hook
    sys.modules["antenv.axon_hooks"] = mod
    antenv.axon_hooks = mod


try:
    _ensure_axon_hooks()
except Exception:
    pass

F32 = mybir.dt.float32
BF16 = mybir.dt.bfloat16
FP8 = mybir.dt.float8e3

B, N, H, D, WF = 2, 1024, 64, 32, 4
NCORES = 8
ROWS_PER_CORE = (B * N) // NCORES  # 256
RB = 16  # rows per j-block
N_JBLK = ROWS_PER_CORE // RB  # 16
N_KCHUNK = N // 128  # 8 s-chunks of 128
FH = WF * H  # 256
Q = ROWS_PER_CORE // 4  # 64 rows per tail quarter
LN_EPS = 1e-6
N_WARM = 7
NEWTON_ITERS = 1
KSPLIT_J = (14, 15)  # j-blocks whose kb DMA is issued per s-chunk
# s-chunks 0..K8-1 stream as fp8 e3m4 (values pre-scaled by KS, with 1/KS
# folded into those chunks' x tiles); chunks K8..7 stream as bf16.
# K8=4 @ KS=2.5 measures 1.0e-2 fro vs the 2e-2 gate (1.6e-2 even if the
# PE flushed fp8 denormals, which it shouldn't).
K8 = 4
K16 = N_KCHUNK - K8
KS = 2.5

# consts blob column layout (f32 words)
BC_WB = 0  # [0:64, 0:512]   wb2: W[d,c] tiled per r
BC_W1 = 512  # [0:65, 512:768] [ln_scale*W1 ; ln_bias@W1+b1]
BC_W2 = 768  # [0:128, 768:896] W2 as [p, fh*64+h]
BC_B2 = 896  # [0:64, 896:960] b2 broadcast over rows
BC_CB = 960  # [0:64, 960:961] conv_bias
BC_W = 968

_NC_CACHE = None
LAST_EXEC_NS = None


def _build_nc(split_waits=True):
    nc = bass.Bass(target_bir_lowering=False)

    kb8 = nc.dram_tensor("kb8", [N_JBLK, 128, K8, RB, D], FP8, kind="ExternalInput")
    kbh = nc.dram_tensor("kbh", [N_JBLK, 128, K16, RB, D], BF16, kind="ExternalInput")
    xcp = nc.dram_tensor("xcp", [128, N_KCHUNK, H], BF16, kind="ExternalInput")
    blob = nc.dram_tensor("blob", [128, BC_W], F32, kind="ExternalInput")
    out = nc.dram_tensor("out", [ROWS_PER_CORE, H], F32, kind="ExternalOutput")

    with tile.TileContext(nc) as tc:
        with (
            tc.tile_pool(name="consts", bufs=1) as consts,
            tc.tile_pool(name="kb8p", bufs=N_JBLK) as kb8_pool,
            tc.tile_pool(name="kbhp", bufs=N_JBLK) as kbh_pool,
            tc.tile_pool(name="mwp", bufs=2) as mw_pool,
            tc.tile_pool(name="tmpp", bufs=2) as tmp_pool,
            tc.tile_pool(name="work", bufs=2) as work,
            tc.tile_pool(name="pmain", bufs=3, space="PSUM") as pmain,
            tc.tile_pool(name="pstatp", bufs=1, space="PSUM") as pstatp,
            tc.tile_pool(name="ptail", bufs=3, space="PSUM") as ptail,
            tc.tile_pool(name="pwarm", bufs=1, space="PSUM") as pwarm,
        ):
            # ---- x + consts blob ride the Scalar engine's DMA queue so the
            # sync queue is kb-only and the stream starts immediately ----
            xc_sb = consts.tile([128, N_KCHUNK, H], BF16)
            nc.scalar.dma_start(out=xc_sb, in_=xcp[:, :, :])
            blob_sb = consts.tile([128, BC_W], F32)
            nc.scalar.dma_start(out=blob_sb, in_=blob[:, :])

            wb_sb = blob_sb[0:H, BC_WB : BC_WB + RB * D]
            w1_sb = blob_sb[0 : H + 1, BC_W1 : BC_W1 + FH]
            w2_sb = blob_sb[:, BC_W2 : BC_W2 + 2 * H]
            b2_sb = blob_sb[0:Q, BC_B2 : BC_B2 + H]
            cb_sb = blob_sb[0:H, BC_CB : BC_CB + 1]

            # ---- the kernel_basis stream: all ~12 MB prefetched; the last
            # two j-blocks arrive per s-chunk so PE can chase the stream ----
            kb_tiles = {}
            for j0 in range(N_JBLK):
                t8 = kb8_pool.tile([128, K8, RB, D], FP8, name=f"kb8_t{j0}", tag="kb8_t")
                t16 = kbh_pool.tile([128, K16, RB, D], BF16, name=f"kbh_t{j0}", tag="kbh_t")
                kb_tiles[j0] = (t8, t16)
                if j0 in KSPLIT_J:
                    for k in range(K8):
                        nc.sync.dma_start(out=t8[:, k, :, :], in_=kb8[j0, :, k, :, :])
                    for k in range(K16):
                        nc.sync.dma_start(out=t16[:, k, :, :], in_=kbh[j0, :, k, :, :])
                else:
                    nc.sync.dma_start(out=t8, in_=kb8[j0, :, :, :, :])
                    nc.sync.dma_start(out=t16, in_=kbh[j0, :, :, :, :])

            # ---- small on-chip constants (GpSimd, idle otherwise) ----
            ones64 = consts.tile([H, 1], F32)
            nc.gpsimd.memset(ones64, 1.0)
            ones1 = consts.tile([1, H], F32)
            nc.gpsimd.memset(ones1, 1.0)
            z_sb = consts.tile([H + 1, Q], F32)
            nc.gpsimd.memset(z_sb[H : H + 1, :], 1.0)
            rp = consts.tile([1, 2 * Q], F32)
            stv = consts.tile([H, 2, ROWS_PER_CORE], F32)  # [a+cb ; (a+cb)^2]

            # ---- PE warm-up on the x tile while kb j0 is in flight (HAM
            # needs ~3us of continuous activity to reach 2.4 GHz) ----
            ps_warm = pwarm.tile([128, RB * D], F32)
            for w in range(N_WARM):
                nc.tensor.matmul(
                    ps_warm[0:H, :],
                    lhsT=xc_sb[:, 0, :],
                    rhs=xc_sb.rearrange("p a b -> p (a b)")[:, 0 : RB * D],
                    start=True,
                    stop=True,
                )

            # persistent LN-stats accumulator: [1, j, (sum(a+cb), sum((a+cb)^2)), r]
            pstat = pstatp.tile([1, N_JBLK, 2, RB], F32)

            # ---- tail pieces per row-quarter, staggered through the j-loop
            # so every engine-queue entry's inputs are long-ready ----
            state = {}

            def t_qt(q):
                sl4 = slice(4 * q, 4 * (q + 1))
                qt = work.tile([1, Q], F32, name=f"qt{q}", tag="qt")
                nc.vector.tensor_scalar(
                    out=qt, in0=pstat[:, sl4, 1, :], scalar1=1.0 / H, scalar2=LN_EPS,
                    op0=mybir.AluOpType.mult, op1=mybir.AluOpType.add,
                )
                mu = work.tile([1, Q], F32, name=f"mu{q}", tag="mu")
                nc.vector.tensor_scalar(
                    out=mu, in0=pstat[:, sl4, 0, :], scalar1=-1.0 / H, scalar2=None,
                    op0=mybir.AluOpType.mult,
                )
                t3 = work.tile([1, Q], F32, name=f"t3_{q}", tag="t3")
                nc.vector.tensor_mul(t3, mu, mu)
                nc.vector.tensor_sub(qt, qt, t3)
                state[("qt", q)] = qt
                state[("mu", q)] = mu

            def t_newton(q):
                qt = state[("qt", q)]
                mu = state[("mu", q)]
                # rsqrt on DVE only (ScalarE's LUT stays pinned on gelu):
                # quake seed via int<->float value casts + Newton steps.
                uf = work.tile([1, Q], F32, name=f"uf{q}", tag="uf")
                nc.vector.tensor_copy(out=uf, in_=qt.bitcast(mybir.dt.int32))
                nc.vector.tensor_scalar(
                    out=uf, in0=uf, scalar1=-0.5, scalar2=float(0x5F3759DF),
                    op0=mybir.AluOpType.mult, op1=mybir.AluOpType.add,
                )
                yi = work.tile([1, Q], mybir.dt.int32, name=f"yi{q}", tag="yi")
                nc.vector.tensor_copy(out=yi, in_=uf)
                y = yi.bitcast(F32)
                t1 = work.tile([1, Q], F32, name=f"t1_{q}", tag="t1")
                for it in range(NEWTON_ITERS):
                    nc.vector.tensor_mul(t1, y, y)
                    nc.vector.tensor_mul(t1, t1, qt)
                    nc.vector.tensor_scalar(
                        out=t1, in0=t1, scalar1=-0.5, scalar2=1.5,
                        op0=mybir.AluOpType.mult, op1=mybir.AluOpType.add,
                    )
                    if it == NEWTON_ITERS - 1:
                        nc.vector.tensor_mul(rp[:, 0:Q], y, t1)
                    else:
                        nc.vector.tensor_mul(y, y, t1)
                nc.vector.tensor_mul(rp[:, Q : 2 * Q], rp[:, 0:Q], mu)

            def t_bc(q):
                ps_bc = ptail.tile([H, 2 * Q], F32, name=f"ps_bc{q}", tag="ps_bc", bufs=1)
                nc.tensor.matmul(ps_bc, lhsT=ones1, rhs=rp, start=True, stop=True)
                nc.vector.tensor_mul(
                    z_sb[0:H, :], stv[:, 0, Q * q : Q * (q + 1)], ps_bc[:, 0:Q]
                )
                nc.vector.tensor_add(z_sb[0:H, :], z_sb[0:H, :], ps_bc[:, Q : 2 * Q])

            def t_mlp_a(q):
                ph = ptail.tile([128, 2, Q], F32, name=f"ph{q}", tag="ph", bufs=1)
                for fh in range(2):
                    nc.tensor.matmul(
                        ph[:, fh, :],
                        lhsT=w1_sb[:, 128 * fh : 128 * (fh + 1)],
                        rhs=z_sb,
                        start=True,
                        stop=True,
                    )
                hT = work.tile([128, 2, Q], F32, name=f"hT{q}", tag="hT")
                nc.scalar.activation(
                    out=hT.rearrange("p a b -> p (a b)"),
                    in_=ph.rearrange("p a b -> p (a b)"),
                    func=mybir.ActivationFunctionType.Gelu_apprx_tanh,
                    bias=0.0,
                    scale=1.0,
                )
                state[("hT", q)] = hT

            def t_mlp_b(q):
                hT = state[("hT", q)]
                po = ptail.tile([Q, H], F32, name=f"po{q}", tag="po", bufs=1)
                for fh in range(2):
                    nc.tensor.matmul(
                        po,
                        lhsT=hT[:, fh, :],
                        rhs=w2_sb[:, H * fh : H * (fh + 1)],
                        start=(fh == 0),
                        stop=(fh == 1),
                    )
                o_sb = work.tile([Q, H], F32, name=f"o_sb{q}", tag="o_sb")
                nc.vector.tensor_add(o_sb, po, b2_sb)
                nc.sync.dma_start(out=out[Q * q : Q * (q + 1), :], in_=o_sb)

            sched = {}
            for q in range(3):
                sched.setdefault(4 * q + 4, []).append(lambda q=q: t_qt(q))
                sched.setdefault(4 * q + 5, []).append(lambda q=q: t_newton(q))
                sched.setdefault(4 * q + 6, []).append(lambda q=q: t_bc(q))
                if 4 * q + 7 < N_JBLK - 1:
                    sched.setdefault(4 * q + 7, []).append(lambda q=q: t_mlp_a(q))
                if 4 * q + 8 < N_JBLK:
                    # runs ahead of the same slot's qt so the out DMA fires asap
                    sched.setdefault(4 * q + 8, []).insert(0, lambda q=q: t_mlp_b(q))

            def stats_mm(j):
                # emitted one j-block late so its DVE-produced inputs are
                # long-ready when the in-order PE queue reaches it
                nc.tensor.matmul(
                    pstat[:, j, :, :],
                    lhsT=ones64,
                    rhs=stv[:, :, RB * j : RB * (j + 1)],
                    start=True,
                    stop=True,
                )

            # ---- main contraction ----
            for j in range(N_JBLK):
                kb8_t, kbh_t = kb_tiles.pop(j)
                if j == N_JBLK - 1:
                    # PE has slack here (stream-gated); pulling this forward
                    # keeps the post-stream PE path minimal
                    stats_mm(j - 1)
                ps = pmain.tile([H, RB * D], F32)
                for k in range(N_KCHUNK):
                    rhs = kb8_t[:, k, :, :] if k < K8 else kbh_t[:, k - K8, :, :]
                    nc.tensor.matmul(
                        ps, lhsT=xc_sb[:, k, :], rhs=rhs,
                        start=(k == 0), stop=(k == N_KCHUNK - 1),
                    )
                if 0 < j < N_JBLK - 1:
                    stats_mm(j - 1)
                mw = mw_pool.tile([H, RB, D], F32)
                nc.vector.tensor_mul(
                    mw.rearrange("p a b -> p (a b)"), ps, wb_sb
                )
                tmp = tmp_pool.tile([H, RB], F32)
                nc.vector.tensor_reduce(
                    out=tmp,
                    in_=mw,
                    axis=mybir.AxisListType.X,
                    op=mybir.AluOpType.add,
                )
                st0 = stv[:, 0, RB * j : RB * (j + 1)]
                nc.vector.tensor_scalar(
                    out=st0, in0=tmp, scalar1=cb_sb, scalar2=None,
                    op0=mybir.AluOpType.add,
                )
                nc.vector.tensor_mul(stv[:, 1, RB * j : RB * (j + 1)], st0, st0)
                for fn in sched.get(j, ()):
                    fn()

            # last quarter after the stream; q2's MLP rides under q3's
            # DVE chain
            stats_mm(N_JBLK - 1)
            t_qt(3)
            t_mlp_a(2)
            t_newton(3)
            t_mlp_b(2)
            t_bc(3)
            t_mlp_a(3)
            t_mlp_b(3)

    if split_waits:
        _split_matmul_waits(nc)
    return nc


def _split_matmul_waits(nc):
    """This walrus build rejects engine instructions carrying more than one
    semaphore wait ("Too many sync wait commands"). Peel all but the last
    wait off onto same-engine NoOps inserted immediately before the
    instruction — NoOps execute in queue order on the same sequencer, so the
    wait semantics are unchanged."""
    f = nc.m.functions[0]
    nop_id = 0
    for blk in f.blocks:
        insts = list(blk.instructions)
        out = []
        changed = False
        for inst in insts:
            si = inst.sync_info
            if (
                si is not None
                and si.on_wait is not None
                and len(si.on_wait) > 1
                and getattr(inst, "engine", None) is not None
            ):
                waits = list(si.on_wait)
                for w in waits[:-1]:
                    nop = mybir.InstNoOp(
                        name=f"I-mmwait-{nop_id}",
                        engine=inst.engine,
                        ins=[],
                        outs=[],
                        sync_info=mybir.SyncInfo(on_wait=[w], on_update=[]),
                    )
                    nop_id += 1
                    out.append(nop)
                inst.sync_info = mybir.SyncInfo(
                    on_wait=[waits[-1]], on_update=list(si.on_update or [])
                )
                changed = True
            out.append(inst)
        if changed:
            blk.instructions = out
    return nc


def _get_nc():
    global _NC_CACHE
    if _NC_CACHE is None:
        _NC_CACHE = _build_nc()
    return _NC_CACHE


def _prep_blob(kernel_W, conv_bias, ln_scale, ln_bias, W1, b1, W2, b2):
    blob = np.zeros((128, BC_W), np.float32)
    # wb2[c, r*D + d] = W[d, c]
    blob[0:H, BC_WB : BC_WB + RB * D] = np.tile(kernel_W.T, (1, RB))
    # ln_scale folded into W1 rows; ln_bias@W1 + b1 as the K=65 bias row
    blob[0:H, BC_W1 : BC_W1 + FH] = W1 * ln_scale[:, None]
    blob[H, BC_W1 : BC_W1 + FH] = ln_bias @ W1 + b1
    blob[:, BC_W2 : BC_W2 + 2 * H] = W2.reshape(2, 128, H).transpose(1, 0, 2).reshape(128, 2 * H)
    blob[0:Q, BC_B2 : BC_B2 + H] = np.broadcast_to(b2, (Q, H))
    blob[0:H, BC_CB] = conv_bias
    return blob


def _prep_x(xb):
    # (N, H) -> (128, k, H) bf16, with s = 128*k + p; the fp8 chunks'
    # kernel_basis is pre-scaled by KS, undone here
    import ml_dtypes

    xs = xb.reshape(N_KCHUNK, 128, H).copy()
    xs[:K8] *= 1.0 / KS
    xh = xs.astype(ml_dtypes.bfloat16)
    return np.ascontiguousarray(xh.transpose(1, 0, 2))


def _prep_kb_shard(shard):
    # shard (256, 1024, 32) -> (j, p, k, r^, d); s-chunks < K8 as e3m4*KS,
    # the rest bf16
    import ml_dtypes

    t = shard.reshape(N_JBLK, RB, N_KCHUNK, 128, D).transpose(0, 3, 2, 1, 4)
    lo = np.ascontiguousarray(t[:, :, :K8] * KS).astype(ml_dtypes.float8_e3m4)
    hi = np.ascontiguousarray(t[:, :, K8:]).astype(ml_dtypes.bfloat16)
    return lo, hi


def kernel(
    x,
    kernel_basis,
    kernel_W,
    conv_bias,
    ln_scale,
    ln_bias,
    W1,
    b1,
    W2,
    b2,
):
    global LAST_EXEC_NS
    x = np.ascontiguousarray(np.asarray(x, np.float32))
    kb = np.ascontiguousarray(np.asarray(kernel_basis, np.float32))
    blob = _prep_blob(
        np.asarray(kernel_W, np.float32),
        np.asarray(conv_bias, np.float32),
        np.asarray(ln_scale, np.float32),
        np.asarray(ln_bias, np.float32),
        np.asarray(W1, np.float32),
        np.asarray(b1, np.float32),
        np.asarray(W2, np.float32),
        np.asarray(b2, np.float32),
    )
    xps = [_prep_x(x[b]) for b in range(B)]

    kbf = kb.reshape(B * N, N, D)
    in_maps = []
    for c in range(NCORES):
        lo, hi = _prep_kb_shard(kbf[c * ROWS_PER_CORE : (c + 1) * ROWS_PER_CORE])
        in_maps.append(dict(kb8=lo, kbh=hi, xcp=xps[c // (NCORES // B)], blob=blob))

    nc = _get_nc()
    trace = bool(os.environ.get("KERNEL_BASS_TRACE"))
    res = run_bass_kernel_spmd(nc, in_maps, core_ids=list(range(NCORES)), trace=trace)
    LAST_EXEC_NS = res.exec_time_ns

    outs = np.concatenate([res.results[c]["out"] for c in range(NCORES)], axis=0)
    return outs.reshape(B, N, H)


# revision 37
# speedup vs baseline: 1.4057x; 1.0960x over previous
"""Trainium2 Bass kernel for nn_ConvBlock (SepGconv + LayerNorm + GELU MLP).

Computes, for full inputs:
    a   = einsum('bsc,brsd,dc->brc', x, kernel_basis, kernel_W) + conv_bias
    a   = LayerNorm(a) * ln_scale + ln_bias          (over channels, eps=1e-6)
    out = gelu_tanh(a @ W1 + b1) @ W2 + b2

Shapes: B=2, N=1024 (R=S=N), H=64, D=32, WF=4.

Sharding: the (B*R)=2048 output rows split into 8 contiguous shards of 256
rows, one per NeuronCore. Each core reads its kernel_basis shard once
(memory-bound), contracts over all S on-chip, and runs the LN/MLP tail
locally. x / weights are replicated.

Perf strategy (HBM-bound at ~360 GB/s per core):
- kernel_basis streams in PURE bf16 (2 B/elem, 16.8 MB/core, ~47 us of bus;
  the gate is rel_err < 2e-2 and bf16 lands ~2.5e-3). All 16 one-MB j-block
  tiles are prefetched into SBUF up front (SBUF is ~26 MB) so the queues
  never wait on buffer recycling.
- dma_start issue costs ~0.6 us of serial sequencer time each, so the kb
  stream is issued alone on the sync queue while x + all small constants
  ride in two packed-blob DMAs issued from the (otherwise idle) Scalar
  engine's queue.
- Each matmul is psum[c, (r,d)] += x[s,c]^T @ kb[s,(r,d)] with N=512
  (16 rows x 32 d), K=128 s-chunk; the d-reduction against kernel_W runs
  on DVE (multiply by W broadcast + free-axis reduce).
- LayerNorm stats are accumulated DURING the stream: per j-block DVE forms
  a+cb and its square, and a tiny ones-matmul drops their channel sums into
  a persistent PSUM tile. Row quarters then only need a short rsqrt/scale/
  MLP chain, staggered through the j-loop; ln_scale/ln_bias are folded into
  W1/b1 on the host so the tail chain is minimal. Only the last quarter's
  chain (~7 us) runs after the stream; the last two kb tiles are DMA'd
  per s-chunk so PE can start on partial tiles.
"""

import os

import numpy as np

import concourse.bass as bass
import concourse.tile as tile
from concourse import mybir
from concourse.bass_utils import run_bass_kernel_spmd


def _ensure_axon_hooks():
    """bass_utils imports antenv.axon_hooks when trace=True under axon; some
    images ship antenv without that module. Register a functional stand-in
    (driving NTFF capture via libaxon_pjrt.so) so tracing works, degrading
    to hook=None (no trace, run still works) if the .so is unavailable."""
    import sys
    import types

    try:
        import antenv.axon_hooks  # noqa: F401

        return
    except ImportError:
        pass
    try:
        import antenv
    except ImportError:
        antenv = types.ModuleType("antenv")
        sys.modules["antenv"] = antenv

    mod = types.ModuleType("antenv.axon_hooks")
    mod._hook = None

    def set_axon_ntff_profile_hook(h):
        mod._hook = h

    def get_axon_ntff_profile_hook():
        if mod._hook is None:
            try:
                from trn_agent_boot.trn_boot import _ntff_profile_via_ctypes

                so_path = "/opt/axon/libaxon_pjrt.so"
                if os.path.exists(so_path):
                    mod._hook = _ntff_profile_via_ctypes(so_path)
            except Exception:
                mod._hook = None
        return mod._hook

    mod.set_axon_ntff_profile_hook = set_axon_ntff_profile_hook
    mod.get_axon_ntff_profile_hook = get_axon_ntff_profile_hook
    sys.modules["antenv.axon_hooks"] = mod
    antenv.axon_hooks = mod


try:
    _ensure_axon_hooks()
except Exception:
    pass

F32 = mybir.dt.float32
BF16 = mybir.dt.bfloat16
FP8 = mybir.dt.float8e3

B, N, H, D, WF = 2, 1024, 64, 32, 4
NCORES = 8
ROWS_PER_CORE = (B * N) // NCORES  # 256
RB = 16  # rows per j-block
N_JBLK = ROWS_PER_CORE // RB  # 16
N_KCHUNK = N // 128  # 8 s-chunks of 128
FH = WF * H  # 256
Q = ROWS_PER_CORE // 4  # 64 rows per tail quarter
LN_EPS = 1e-6
N_WARM = 9
NEWTON_ITERS = 1
# s-chunks 0..K8-1 stream as fp8 e3m4 (values pre-scaled by KS, with 1/KS
# folded into those chunks' x tiles); chunks K8..7 stream as bf16.
# K8=4 @ KS=2.5 measures 1.0e-2 fro vs the 2e-2 gate (1.6e-2 even if the
# PE flushed fp8 denormals, which it shouldn't).
K8 = 4
K16 = N_KCHUNK - K8
KS = 2.5

# consts blob column layout (f32 words)
BC_WB = 0  # [0:64, 0:512]   wb2: W[d,c] tiled per r
BC_B2 = 512  # [0:64, 512:576] b2 broadcast over rows
BC_CB = 576  # [0:64, 576:577] conv_bias
BC_W = 584
# bf16 blob (MLP weights run in bf16: fp32 matmuls cost 2 HW passes)
BH_W1 = 0  # [0:65, 0:256]   [ln_scale*W1 ; ln_bias@W1+b1]
BH_W2 = 256  # [0:128, 256:384] W2 as [p, fh*64+h]
BH_W = 384

_NC_CACHE = None
LAST_EXEC_NS = None


def _build_nc(split_waits=True):
    nc = bass.Bass(target_bir_lowering=False)

    kb8 = nc.dram_tensor("kb8", [N_JBLK, 128, K8, RB, D], FP8, kind="ExternalInput")
    kbh = nc.dram_tensor("kbh", [N_JBLK, 128, K16, RB, D], BF16, kind="ExternalInput")
    xcp = nc.dram_tensor("xcp", [128, N_KCHUNK, H], BF16, kind="ExternalInput")
    blob = nc.dram_tensor("blob", [128, BC_W], F32, kind="ExternalInput")
    blob16 = nc.dram_tensor("blob16", [128, BH_W], BF16, kind="ExternalInput")
    out = nc.dram_tensor("out", [ROWS_PER_CORE, H], F32, kind="ExternalOutput")

    with tile.TileContext(nc) as tc:
        with (
            tc.tile_pool(name="consts", bufs=1) as consts,
            tc.tile_pool(name="kb8p", bufs=N_JBLK) as kb8_pool,
            tc.tile_pool(name="kbhp", bufs=N_JBLK) as kbh_pool,
            tc.tile_pool(name="mwp", bufs=2) as mw_pool,
            tc.tile_pool(name="tmpp", bufs=2) as tmp_pool,
            tc.tile_pool(name="work", bufs=2) as work,
            tc.tile_pool(name="pmain", bufs=3, space="PSUM") as pmain,
            tc.tile_pool(name="pstatp", bufs=1, space="PSUM") as pstatp,
            tc.tile_pool(name="ptail", bufs=3, space="PSUM") as ptail,
            tc.tile_pool(name="pwarm", bufs=1, space="PSUM") as pwarm,
        ):
            # ---- x + consts blob ride the Scalar engine's DMA queue so the
            # sync queue is kb-only and the stream starts immediately ----
            xc_sb = consts.tile([128, N_KCHUNK, H], BF16)
            nc.scalar.dma_start(out=xc_sb, in_=xcp[:, :, :])
            blob_sb = consts.tile([128, BC_W], F32)
            nc.scalar.dma_start(out=blob_sb, in_=blob[:, :])
            blob16_sb = consts.tile([128, BH_W], BF16)
            nc.scalar.dma_start(out=blob16_sb, in_=blob16[:, :])

            wb_sb = blob_sb[0:H, BC_WB : BC_WB + RB * D]
            b2_sb = blob_sb[0:Q, BC_B2 : BC_B2 + H]
            cb_sb = blob_sb[0:H, BC_CB : BC_CB + 1]
            w1_sb = blob16_sb[0 : H + 1, BH_W1 : BH_W1 + FH]
            w2_sb = blob16_sb[:, BH_W2 : BH_W2 + 2 * H]

            # ---- the kernel_basis stream: all ~12 MB prefetched; the last
            # j-block arrives in 3 pieces so PE can chase the stream (finer
            # splits trickle: sub-2KB descriptors run well below bus rate) ----
            kb_tiles = {}
            for j0 in range(N_JBLK):
                t8 = kb8_pool.tile([128, K8, RB, D], FP8, name=f"kb8_t{j0}", tag="kb8_t")
                t16 = kbh_pool.tile([128, K16, RB, D], BF16, name=f"kbh_t{j0}", tag="kbh_t")
                kb_tiles[j0] = (t8, t16)
                nc.sync.dma_start(out=t8, in_=kb8[j0, :, :, :, :])
                if j0 == N_JBLK - 1:
                    half = K16 // 2
                    nc.sync.dma_start(
                        out=t16[:, 0:half, :, :], in_=kbh[j0, :, 0:half, :, :]
                    )
                    nc.sync.dma_start(
                        out=t16[:, half:, :, :], in_=kbh[j0, :, half:, :, :]
                    )
                else:
                    nc.sync.dma_start(out=t16, in_=kbh[j0, :, :, :, :])

            # ---- small on-chip constants (GpSimd, idle otherwise) ----
            wtile = consts.tile([128, RB * D], BF16)
            nc.gpsimd.memset(wtile, 0.25)
            ones64 = consts.tile([H, 1], F32)
            nc.gpsimd.memset(ones64, 1.0)
            ones1 = consts.tile([1, H], F32)
            nc.gpsimd.memset(ones1, 1.0)
            z_sb = consts.tile([H + 1, Q], BF16)
            nc.gpsimd.memset(z_sb[H : H + 1, :], 1.0)
            rp = consts.tile([1, 2 * Q], F32)
            stv = consts.tile([H, 2, ROWS_PER_CORE], F32)  # [a+cb ; (a+cb)^2]

            # ---- PE warm-up on a memset tile: the ~3us HAM ramp to 2.4 GHz
            # runs while x is still in flight, so j0 starts at full speed ----
            ps_warm = pwarm.tile([128, RB * D], F32)
            for w in range(N_WARM):
                nc.tensor.matmul(
                    ps_warm[0:H, :],
                    lhsT=wtile[:, 0:H],
                    rhs=wtile,
                    start=True,
                    stop=True,
                )

            # persistent LN-stats accumulator: [1, j, (sum(a+cb), sum((a+cb)^2)), r]
            pstat = pstatp.tile([1, N_JBLK, 2, RB], F32)

            # ---- tail pieces per row-quarter, staggered through the j-loop
            # so every engine-queue entry's inputs are long-ready ----
            state = {}

            def t_qt(q):
                sl4 = slice(4 * q, 4 * (q + 1))
                qt = work.tile([1, Q], F32, name=f"qt{q}", tag="qt")
                nc.vector.tensor_scalar(
                    out=qt, in0=pstat[:, sl4, 1, :], scalar1=1.0 / H, scalar2=LN_EPS,
                    op0=mybir.AluOpType.mult, op1=mybir.AluOpType.add,
                )
                mu = work.tile([1, Q], F32, name=f"mu{q}", tag="mu")
                nc.vector.tensor_scalar(
                    out=mu, in0=pstat[:, sl4, 0, :], scalar1=-1.0 / H, scalar2=None,
                    op0=mybir.AluOpType.mult,
                )
                t3 = work.tile([1, Q], F32, name=f"t3_{q}", tag="t3")
                nc.vector.tensor_mul(t3, mu, mu)
                nc.vector.tensor_sub(qt, qt, t3)
                state[("qt", q)] = qt
                state[("mu", q)] = mu

            def t_newton(q):
                qt = state[("qt", q)]
                mu = state[("mu", q)]
                # rsqrt on DVE only (ScalarE's LUT stays pinned on gelu):
                # quake seed via int<->float value casts + Newton steps.
                uf = work.tile([1, Q], F32, name=f"uf{q}", tag="uf")
                nc.vector.tensor_copy(out=uf, in_=qt.bitcast(mybir.dt.int32))
                nc.vector.tensor_scalar(
                    out=uf, in0=uf, scalar1=-0.5, scalar2=float(0x5F3759DF),
                    op0=mybir.AluOpType.mult, op1=mybir.AluOpType.add,
                )
                yi = work.tile([1, Q], mybir.dt.int32, name=f"yi{q}", tag="yi")
                nc.vector.tensor_copy(out=yi, in_=uf)
                y = yi.bitcast(F32)
                t1 = work.tile([1, Q], F32, name=f"t1_{q}", tag="t1")
                for it in range(NEWTON_ITERS):
                    nc.vector.tensor_mul(t1, y, y)
                    nc.vector.tensor_mul(t1, t1, qt)
                    nc.vector.tensor_scalar(
                        out=t1, in0=t1, scalar1=-0.5, scalar2=1.5,
                        op0=mybir.AluOpType.mult, op1=mybir.AluOpType.add,
                    )
                    if it == NEWTON_ITERS - 1:
                        nc.vector.tensor_mul(rp[:, 0:Q], y, t1)
                    else:
                        nc.vector.tensor_mul(y, y, t1)
                nc.vector.tensor_mul(rp[:, Q : 2 * Q], rp[:, 0:Q], mu)

            def t_bc(q):
                ps_bc = ptail.tile([H, 2 * Q], F32, name=f"ps_bc{q}", tag="ps_bc", bufs=1)
                nc.tensor.matmul(ps_bc, lhsT=ones1, rhs=rp, start=True, stop=True)
                nc.vector.tensor_mul(
                    z_sb[0:H, :], stv[:, 0, Q * q : Q * (q + 1)], ps_bc[:, 0:Q]
                )
                nc.vector.tensor_add(z_sb[0:H, :], z_sb[0:H, :], ps_bc[:, Q : 2 * Q])

            def t_mlp_a(q):
                ph = ptail.tile([128, 2, Q], F32, name=f"ph{q}", tag="ph", bufs=1)
                for fh in range(2):
                    nc.tensor.matmul(
                        ph[:, fh, :],
                        lhsT=w1_sb[:, 128 * fh : 128 * (fh + 1)],
                        rhs=z_sb,
                        start=True,
                        stop=True,
                    )
                hT = work.tile([128, 2, Q], BF16, name=f"hT{q}", tag="hT")
                nc.scalar.activation(
                    out=hT.rearrange("p a b -> p (a b)"),
                    in_=ph.rearrange("p a b -> p (a b)"),
                    func=mybir.ActivationFunctionType.Gelu_apprx_tanh,
                    bias=0.0,
                    scale=1.0,
                )
                state[("hT", q)] = hT

            def t_mlp_b(q):
                hT = state[("hT", q)]
                po = ptail.tile([Q, H], F32, name=f"po{q}", tag="po", bufs=1)
                for fh in range(2):
                    nc.tensor.matmul(
                        po,
                        lhsT=hT[:, fh, :],
                        rhs=w2_sb[:, H * fh : H * (fh + 1)],
                        start=(fh == 0),
                        stop=(fh == 1),
                    )
                o_sb = work.tile([Q, H], F32, name=f"o_sb{q}", tag="o_sb")
                nc.vector.tensor_add(o_sb, po, b2_sb)
                nc.sync.dma_start(out=out[Q * q : Q * (q + 1), :], in_=o_sb)

            sched = {}
            for q in range(3):
                sched.setdefault(4 * q + 4, []).append(lambda q=q: t_qt(q))
                sched.setdefault(4 * q + 5, []).append(lambda q=q: t_newton(q))
                sched.setdefault(4 * q + 6, []).append(lambda q=q: t_bc(q))
                if 4 * q + 7 < N_JBLK - 1:
                    sched.setdefault(4 * q + 7, []).append(lambda q=q: t_mlp_a(q))
                if 4 * q + 8 < N_JBLK:
                    # runs ahead of the same slot's qt so the out DMA fires asap
                    sched.setdefault(4 * q + 8, []).insert(0, lambda q=q: t_mlp_b(q))

            def stats_mm(j):
                # emitted one j-block late so its DVE-produced inputs are
                # long-ready when the in-order PE queue reaches it
                nc.tensor.matmul(
                    pstat[:, j, :, :],
                    lhsT=ones64,
                    rhs=stv[:, :, RB * j : RB * (j + 1)],
                    start=True,
                    stop=True,
                )

            # ---- main contraction ----
            for j in range(N_JBLK):
                kb8_t, kbh_t = kb_tiles.pop(j)
                if j == N_JBLK - 1:
                    # PE has slack here (stream-gated); pulling this forward
                    # keeps the post-stream PE path minimal
                    stats_mm(j - 1)
                ps = pmain.tile([H, RB * D], F32)
                for k in range(N_KCHUNK):
                    rhs = kb8_t[:, k, :, :] if k < K8 else kbh_t[:, k - K8, :, :]
                    nc.tensor.matmul(
                        ps, lhsT=xc_sb[:, k, :], rhs=rhs,
                        start=(k == 0), stop=(k == N_KCHUNK - 1),
                    )
                if 0 < j < N_JBLK - 1:
                    stats_mm(j - 1)
                mw = mw_pool.tile([H, RB, D], F32)
                nc.vector.tensor_mul(
                    mw.rearrange("p a b -> p (a b)"), ps, wb_sb
                )
                tmp = tmp_pool.tile([H, RB], F32)
                nc.vector.tensor_reduce(
                    out=tmp,
                    in_=mw,
                    axis=mybir.AxisListType.X,
                    op=mybir.AluOpType.add,
                )
                st0 = stv[:, 0, RB * j : RB * (j + 1)]
                nc.vector.tensor_scalar(
                    out=st0, in0=tmp, scalar1=cb_sb, scalar2=None,
                    op0=mybir.AluOpType.add,
                )
                nc.vector.tensor_mul(stv[:, 1, RB * j : RB * (j + 1)], st0, st0)
                for fn in sched.get(j, ()):
                    fn()

            # last quarter after the stream; q2's MLP rides under q3's
            # DVE chain
            stats_mm(N_JBLK - 1)
            t_qt(3)
            t_mlp_a(2)
            t_newton(3)
            t_mlp_b(2)
            t_bc(3)
            t_mlp_a(3)
            t_mlp_b(3)

    if split_waits:
        _split_matmul_waits(nc)
    return nc


def _split_matmul_waits(nc):
    """This walrus build rejects engine instructions carrying more than one
    semaphore wait ("Too many sync wait commands"). Peel all but the last
    wait off onto same-engine NoOps inserted immediately before the
    instruction — NoOps execute in queue order on the same sequencer, so the
    wait semantics are unchanged."""
    f = nc.m.functions[0]
    nop_id = 0
    for blk in f.blocks:
        insts = list(blk.instructions)
        out = []
        changed = False
        for inst in insts:
            si = inst.sync_info
            if (
                si is not None
                and si.on_wait is not None
                and len(si.on_wait) > 1
                and getattr(inst, "engine", None) is not None
            ):
                waits = list(si.on_wait)
                for w in waits[:-1]:
                    nop = mybir.InstNoOp(
                        name=f"I-mmwait-{nop_id}",
                        engine=inst.engine,
                        ins=[],
                        outs=[],
                        sync_info=mybir.SyncInfo(on_wait=[w], on_update=[]),
                    )
                    nop_id += 1
                    out.append(nop)
                inst.sync_info = mybir.SyncInfo(
                    on_wait=[waits[-1]], on_update=list(si.on_update or [])
                )
                changed = True
            out.append(inst)
        if changed:
            blk.instructions = out
    return nc


def _get_nc():
    global _NC_CACHE
    if _NC_CACHE is None:
        _NC_CACHE = _build_nc()
    return _NC_CACHE


def _prep_blob(kernel_W, conv_bias, ln_scale, ln_bias, W1, b1, W2, b2):
    import ml_dtypes

    blob = np.zeros((128, BC_W), np.float32)
    # wb2[c, r*D + d] = W[d, c]
    blob[0:H, BC_WB : BC_WB + RB * D] = np.tile(kernel_W.T, (1, RB))
    blob[0:Q, BC_B2 : BC_B2 + H] = np.broadcast_to(b2, (Q, H))
    blob[0:H, BC_CB] = conv_bias

    blob16 = np.zeros((128, BH_W), ml_dtypes.bfloat16)
    # ln_scale folded into W1 rows; ln_bias@W1 + b1 as the K=65 bias row
    blob16[0:H, BH_W1 : BH_W1 + FH] = W1 * ln_scale[:, None]
    blob16[H, BH_W1 : BH_W1 + FH] = ln_bias @ W1 + b1
    blob16[:, BH_W2 : BH_W2 + 2 * H] = (
        W2.reshape(2, 128, H).transpose(1, 0, 2).reshape(128, 2 * H)
    )
    return blob, blob16


def _prep_x(xb):
    # (N, H) -> (128, k, H) bf16, with s = 128*k + p; the fp8 chunks'
    # kernel_basis is pre-scaled by KS, undone here
    import ml_dtypes

    xs = xb.reshape(N_KCHUNK, 128, H).copy()
    xs[:K8] *= 1.0 / KS
    xh = xs.astype(ml_dtypes.bfloat16)
    return np.ascontiguousarray(xh.transpose(1, 0, 2))


def _prep_kb_shard(shard):
    # shard (256, 1024, 32) -> (j, p, k, r^, d); s-chunks < K8 as e3m4*KS,
    # the rest bf16
    import ml_dtypes

    t = shard.reshape(N_JBLK, RB, N_KCHUNK, 128, D).transpose(0, 3, 2, 1, 4)
    lo = np.ascontiguousarray(t[:, :, :K8] * KS).astype(ml_dtypes.float8_e3m4)
    hi = np.ascontiguousarray(t[:, :, K8:]).astype(ml_dtypes.bfloat16)
    return lo, hi


def kernel(
    x,
    kernel_basis,
    kernel_W,
    conv_bias,
    ln_scale,
    ln_bias,
    W1,
    b1,
    W2,
    b2,
):
    global LAST_EXEC_NS
    x = np.ascontiguousarray(np.asarray(x, np.float32))
    kb = np.ascontiguousarray(np.asarray(kernel_basis, np.float32))
    blob, blob16 = _prep_blob(
        np.asarray(kernel_W, np.float32),
        np.asarray(conv_bias, np.float32),
        np.asarray(ln_scale, np.float32),
        np.asarray(ln_bias, np.float32),
        np.asarray(W1, np.float32),
        np.asarray(b1, np.float32),
        np.asarray(W2, np.float32),
        np.asarray(b2, np.float32),
    )
    xps = [_prep_x(x[b]) for b in range(B)]

    kbf = kb.reshape(B * N, N, D)
    in_maps = []
    for c in range(NCORES):
        lo, hi = _prep_kb_shard(kbf[c * ROWS_PER_CORE : (c + 1) * ROWS_PER_CORE])
        in_maps.append(
            dict(kb8=lo, kbh=hi, xcp=xps[c // (NCORES // B)], blob=blob, blob16=blob16)
        )

    nc = _get_nc()
    trace = bool(os.environ.get("KERNEL_BASS_TRACE"))
    res = run_bass_kernel_spmd(nc, in_maps, core_ids=list(range(NCORES)), trace=trace)
    LAST_EXEC_NS = res.exec_time_ns

    outs = np.concatenate([res.results[c]["out"] for c in range(NCORES)], axis=0)
    return outs.reshape(B, N, H)
